# revision 9
# baseline (speedup 1.0000x reference)
"""Trainium2 Bass kernel for nn_GATv2Layer4View (GAT message passing + inter-view MHA).

Self-contained: kernel(**inputs) -> np.ndarray [2, 4, 10000, 128] float32.

Math notes (faithful to the reference):
  scores[e,h] = mean_b(s_src[b, src[e], h] + s_dst[b, dst[e], h])  -- node-separable!
  w = softmax(scores, axis=0)  (over ALL edges, per head)
      => w[e,h] = ea[src[e],h] * eb[dst[e],h] / Z[h]
         with ea = exp(ms_src), eb = exp(ms_dst), Z = sum_e ea[src]*eb[dst]
  out[b,d,:] = sum_{e: dst=d} w[e] (x) h[b, src[e]]
             = (eb[d]/Z) (x) sum_{e: dst=d} (ea[src[e]] (x) h[b, src[e]])
  So the edge aggregation is a pure unweighted gather+scatter-add of
  rows [ea*h | ea] (132 f32), done as dma_gather + one-hot matmul into PSUM.

Sharding: launch1/2 data-parallel over B*V=8 (one (b,v) per core, edges
replicated); launch3 (inter-view MHA) sharded over (b, node-quarter).
"""

import math
import numpy as np

import concourse.bass as bass
import concourse.bacc as bacc
import concourse.mybir as mybir
import concourse.tile as tile
import concourse.bass_isa as bass_isa
from concourse.bass_utils import run_bass_kernel_spmd
from concourse.masks import make_identity

P = 128
NCORES = 8
B, V, N, FIN = 2, 4, 10000, 64
H, F = 4, 32
D = H * F                      # 128
E_RAW = 160000
E = E_RAW + N                  # with self-loops
NEG_SLOPE = 0.2
TBL_COLS = 192                 # 768B rows (multiple of 256B required by dma_gather)
USE_COLS = D + H               # 132: [ea*h (128) | ea (4)]
GATHER_GROUP = 16              # chunks (of 128 edges) per dma_gather
N_TILES = math.ceil(N / P)     # 79
LAST_NT = N - (N_TILES - 1) * P

FP32 = mybir.dt.float32
I16 = mybir.dt.int16
I32 = mybir.dt.int32


# --------------------------------------------------------------------------
# host-side edge preprocessing
# --------------------------------------------------------------------------
class EdgePlan:
    pass


def prep_edges(edge_index: np.ndarray) -> EdgePlan:
    ei = np.asarray(edge_index)
    src = np.concatenate([ei[0].astype(np.int64), np.arange(N)])
    dst = np.concatenate([ei[1].astype(np.int64), np.arange(N)])
    order = np.argsort(dst, kind="stable")
    ss, ds = src[order], dst[order]

    bounds = np.searchsorted(ds, np.arange(N_TILES + 1) * P)
    gidx_parts, rel_parts, chunk_tile = [], [], []
    for t in range(N_TILES):
        s_t = ss[bounds[t]:bounds[t + 1]]
        d_t = ds[bounds[t]:bounds[t + 1]] - t * P
        pad = (-len(s_t)) % P
        if pad:
            s_t = np.concatenate([s_t, np.full(pad, N, np.int64)])
            d_t = np.concatenate([d_t, np.full(pad, 1000, np.int64)])
        nchunk = len(s_t) // P
        gidx_parts.append(s_t)
        rel_parts.append(d_t)
        chunk_tile.extend([t] * nchunk)

    gidx = np.concatenate(gidx_parts).astype(np.int16)     # [n_chunks*128]
    rel = np.concatenate(rel_parts).astype(np.float32)
    n_chunks = len(chunk_tile)

    plan = EdgePlan()
    plan.n_chunks = n_chunks
    plan.chunk_tile = chunk_tile
    plan.idx16 = np.ascontiguousarray(gidx.reshape(-1, 16).T)          # [16, n_chunks*8]
    plan.rel = np.ascontiguousarray(rel.reshape(-1, P).T)              # [128, n_chunks]
    # start/stop flags for PSUM accumulation per node-tile
    plan.first = [i == 0 or chunk_tile[i - 1] != chunk_tile[i] for i in range(n_chunks)]
    plan.last = [i == n_chunks - 1 or chunk_tile[i + 1] != chunk_tile[i]
                 for i in range(n_chunks)]
    # gather groups of consecutive chunks
    plan.groups = []
    c = 0
    while c < n_chunks:
        m = min(GATHER_GROUP, n_chunks - c)
        plan.groups.append((c, m))
        c += m
    plan.key = (n_chunks, tuple(chunk_tile))
    return plan


# --------------------------------------------------------------------------
# launch 1: per-core (b,v): hT = W @ x^T  [128, N];  sT = att-scores [8, N]
# --------------------------------------------------------------------------
def build_launch1():
    nc = bacc.Bacc("TRN2", target_bir_lowering=False, debug=False,
                   num_devices=NCORES)
    xT = nc.dram_tensor("xT", [FIN, N], FP32, kind="ExternalInput")
    wT = nc.dram_tensor("wT", [FIN, D], FP32, kind="ExternalInput")
    att2T = nc.dram_tensor("att2T", [P, 2], FP32, kind="ExternalInput")
    indsrc = nc.dram_tensor("indsrc", [P, 8], FP32, kind="ExternalInput")
    inddst = nc.dram_tensor("inddst", [P, 8], FP32, kind="ExternalInput")
    hT_out = nc.dram_tensor("hT", [P, N], FP32, kind="ExternalOutput")
    sT_out = nc.dram_tensor("sT", [8, N], FP32, kind="ExternalOutput")

    with tile.TileContext(nc) as tc:
        with tc.tile_pool(name="one", bufs=1) as one, \
             tc.tile_pool(name="sb", bufs=3) as sb, \
             tc.tile_pool(name="ps", bufs=3, space="PSUM") as ps, \
             tc.tile_pool(name="ps2", bufs=2, space="PSUM") as ps2:
            xT_sb = one.tile([FIN, N], FP32)
            nc.sync.dma_start(xT_sb[:], xT.ap()[:])
            wT_sb = one.tile([FIN, D], FP32)
            nc.sync.dma_start(wT_sb[:], wT.ap()[:])
            att_sb = one.tile([P, 2], FP32)
            nc.sync.dma_start(att_sb[:], att2T.ap()[:])
            ind_sb = one.tile([P, 16], FP32)
            nc.sync.dma_start(ind_sb[:, 0:8], indsrc.ap()[:])
            nc.sync.dma_start(ind_sb[:, 8:16], inddst.ap()[:])

            hT_sb = one.tile([P, N], FP32)
            sT_sb = one.tile([8, N], FP32)

            for t in range(N_TILES):
                n0 = t * P
                nt = min(P, N - n0)
                h_ps = ps.tile([P, P], FP32, tag="h")
                nc.tensor.matmul(h_ps[:, :nt], wT_sb[:], xT_sb[:, n0:n0 + nt],
                                 start=True, stop=True)
                t1 = sb.tile([P, P], FP32, tag="t1")
                nc.scalar.mul(t1[:, :nt], h_ps[:, :nt], NEG_SLOPE)
                hl = sb.tile([P, P], FP32, tag="hl")
                nc.vector.tensor_tensor(out=hl[:, :nt], in0=h_ps[:, :nt],
                                        in1=t1[:, :nt], op=mybir.AluOpType.max)
                nc.vector.tensor_copy(hT_sb[:, n0:n0 + nt], h_ps[:, :nt])
                psrc = sb.tile([P, P], FP32, tag="psrc")
                nc.vector.tensor_scalar_mul(psrc[:, :nt], hl[:, :nt], att_sb[:, 0:1])
                pdst = sb.tile([P, P], FP32, tag="pdst")
                nc.vector.tensor_scalar_mul(pdst[:, :nt], hl[:, :nt], att_sb[:, 1:2])
                s_ps = ps2.tile([8, P], FP32, tag="s")
                nc.tensor.matmul(s_ps[:, :nt], ind_sb[:, 0:8], psrc[:, :nt],
                                 start=True, stop=False)
                nc.tensor.matmul(s_ps[:, :nt], ind_sb[:, 8:16], pdst[:, :nt],
                                 start=False, stop=True)
                nc.vector.tensor_copy(sT_sb[:, n0:n0 + nt], s_ps[:, :nt])

            nc.sync.dma_start(hT_out.ap()[:], hT_sb[:])
            nc.sync.dma_start(sT_out.ap()[:], sT_sb[:])
    nc.compile()
    return nc


# --------------------------------------------------------------------------
# launch 2: per-core (b,v): GAT edge aggregation -> gatT [128, N]
# --------------------------------------------------------------------------
def build_launch2(plan: EdgePlan):
    n_chunks = plan.n_chunks
    idx_cols = plan.idx16.shape[1]

    nc = bacc.Bacc("TRN2", target_bir_lowering=False, debug=False,
                   num_devices=NCORES)
    hT_in = nc.dram_tensor("hT", [P, N], FP32, kind="ExternalInput")
    s_in = nc.dram_tensor("s_all", [N, 64], FP32, kind="ExternalInput")
    idx_in = nc.dram_tensor("idx16", [16, idx_cols], I16, kind="ExternalInput")
    rel_in = nc.dram_tensor("rel", [P, n_chunks], FP32, kind="ExternalInput")
    gatT_out = nc.dram_tensor("gatT", [P, N], FP32, kind="ExternalOutput")

    with tile.TileContext(nc) as tc:
        with tc.tile_pool(name="one", bufs=1) as one, \
             tc.tile_pool(name="big", bufs=1) as big, \
             tc.tile_pool(name="sb", bufs=3) as sb, \
             tc.tile_pool(name="gp", bufs=3) as gp, \
             tc.tile_pool(name="ps", bufs=2, space="PSUM") as ps, \
             tc.tile_pool(name="acc", bufs=2, space="PSUM") as accp, \
             tc.tile_pool(name="dram", bufs=1, space="DRAM") as dram:

            table = dram.tile([N + 1, TBL_COLS], FP32)

            identity = one.tile([P, P], FP32)
            make_identity(nc, identity[:])
            iota_i = one.tile([P, P], I32)
            nc.gpsimd.iota(iota_i[:], [[1, P]], channel_multiplier=0)
            iota_f = one.tile([P, P], FP32)
            nc.vector.tensor_copy(iota_f[:], iota_i[:])

            idx_sb = one.tile([P, idx_cols], I16)
            for r in range(8):
                nc.sync.dma_start(idx_sb[16 * r:16 * (r + 1), :], idx_in.ap()[:])
            rel_sb = one.tile([P, n_chunks], FP32)
            nc.sync.dma_start(rel_sb[:], rel_in.ap()[:])

            # hT resident; later reused (same tag) for gatT accumulation
            hT_sb = big.tile([P, N], FP32, tag="bigbuf")
            nc.sync.dma_start(hT_sb[:], hT_in.ap()[:])
            s_sb = one.tile([P, N_TILES * 64], FP32)
            nfull = (N_TILES - 1) * P
            nc.sync.dma_start(
                s_sb[:, 0:(N_TILES - 1) * 64].rearrange(
                    "p (t c) -> p t c", c=64),
                s_in.ap()[0:nfull, :].rearrange("(t p) c -> p t c", p=P),
            )
            nc.sync.dma_start(
                s_sb[0:N - nfull, (N_TILES - 1) * 64:N_TILES * 64],
                s_in.ap()[nfull:N, :],
            )

            ee_all = one.tile([P, N_TILES * 8], FP32)   # [ea(4) eb(4)] per tile
            agg_all = one.tile([P, N_TILES * D], FP32)
            zacc = one.tile([P, H], FP32)
            nc.vector.memset(zacc[:], 0.0)

            zrow = one.tile([1, USE_COLS], FP32)
            nc.vector.memset(zrow[:], 0.0)
            nc.sync.dma_start(table[N:N + 1, 0:USE_COLS], zrow[:])

            # ---- phase A: build table rows [ea*h | ea] ----
            for t in range(N_TILES):
                n0 = t * P
                nt = min(P, N - n0)
                h_ps = ps.tile([P, P], FP32, tag="hT")
                nc.tensor.transpose(h_ps[:nt, :], hT_sb[:, n0:n0 + nt], identity[:])
                ms = sb.tile([P, 8], FP32, tag="ms")
                nc.vector.tensor_reduce(
                    out=ms[:nt, :],
                    in_=s_sb[:nt, t * 64:(t + 1) * 64].rearrange(
                        "p (c k) -> p c k", c=8),
                    axis=mybir.AxisListType.X, op=mybir.AluOpType.add)
                ee = ee_all[:nt, t * 8:(t + 1) * 8]
                nc.scalar.activation(ee, ms[:nt, :],
                                     mybir.ActivationFunctionType.Exp,
                                     scale=1.0 / NCORES)
                packed = sb.tile([P, USE_COLS], FP32, tag="packed")
                ea = ee_all[:nt, t * 8:t * 8 + 4]
                nc.vector.tensor_tensor(
                    out=packed[:nt, 0:D].rearrange("p (h f) -> p h f", h=H),
                    in0=h_ps[:nt, :].rearrange("p (h f) -> p h f", h=H),
                    in1=ea[:, :, None].to_broadcast([nt, H, F]),
                    op=mybir.AluOpType.mult)
                nc.vector.tensor_copy(packed[:nt, D:USE_COLS], ea)
                nc.sync.dma_start(table[n0:n0 + nt, 0:USE_COLS], packed[:nt, :])

            # ---- phase B: gather + one-hot scatter matmul ----
            acc_ps = None
            for (c0, m) in plan.groups:
                g = gp.tile([P, GATHER_GROUP, TBL_COLS], FP32, tag="g")
                nc.gpsimd.dma_gather(
                    out_ap=g[:, :m, :],
                    in_ap=table[:],
                    idxs_ap=idx_sb[:, c0 * 8:(c0 + m) * 8],
                    num_idxs=m * P,
                    num_idxs_reg=m * P,
                    elem_size=TBL_COLS,
                    single_packet=False,
                )
                for j in range(m):
                    c = c0 + j
                    t = plan.chunk_tile[c]
                    nt = min(P, N - t * P)
                    if plan.first[c]:
                        acc_ps = accp.tile([P, USE_COLS], FP32, tag="acc")
                    S = sb.tile([P, P], FP32, tag="S")
                    nc.vector.tensor_scalar(
                        out=S[:, :nt], in0=iota_f[:, :nt],
                        scalar1=rel_sb[:, c:c + 1], scalar2=None,
                        op0=mybir.AluOpType.is_equal)
                    nc.tensor.matmul(acc_ps[:nt, :], S[:, :nt], g[:, j, 0:USE_COLS],
                                     start=plan.first[c], stop=plan.last[c])
                    if plan.last[c]:
                        nc.vector.tensor_copy(
                            agg_all[:nt, t * D:(t + 1) * D], acc_ps[:nt, 0:D])
                        zp = sb.tile([P, H], FP32, tag="zp")
                        nc.vector.tensor_tensor(
                            out=zp[:nt, :], in0=acc_ps[:nt, D:USE_COLS],
                            in1=ee_all[:nt, t * 8 + 4:t * 8 + 8],
                            op=mybir.AluOpType.mult)
                        nc.vector.tensor_tensor(
                            out=zacc[:nt, :], in0=zacc[:nt, :], in1=zp[:nt, :],
                            op=mybir.AluOpType.add)

            # ---- Z and finalize ----
            zred = one.tile([P, H], FP32)
            nc.gpsimd.partition_all_reduce(zred[:], zacc[:], channels=P,
                                           reduce_op=bass_isa.ReduceOp.add)
            rz = one.tile([P, H], FP32)
            nc.vector.reciprocal(rz[:], zred[:])

            gatT_sb = big.tile([P, N], FP32, tag="bigbuf")
            for t in range(N_TILES):
                n0 = t * P
                nt = min(P, N - n0)
                ebz = sb.tile([P, H], FP32, tag="ebz")
                nc.vector.tensor_tensor(
                    out=ebz[:nt, :], in0=ee_all[:nt, t * 8 + 4:t * 8 + 8],
                    in1=rz[:nt, :], op=mybir.AluOpType.mult)
                om = sb.tile([P, D], FP32, tag="om")
                nc.vector.tensor_tensor(
                    out=om[:nt, :].rearrange("p (h f) -> p h f", h=H),
                    in0=agg_all[:nt, t * D:(t + 1) * D].rearrange(
                        "p (h f) -> p h f", h=H),
                    in1=ebz[:nt, :, None].to_broadcast([nt, H, F]),
                    op=mybir.AluOpType.mult)
                o_ps = ps.tile([P, P], FP32, tag="oT")
                nc.tensor.transpose(o_ps[:, :nt], om[:nt, :], identity[:nt, :nt])
                nc.vector.tensor_copy(gatT_sb[:, n0:n0 + nt], o_ps[:, :nt])

            nc.sync.dma_start(gatT_out.ap()[:], gatT_sb[:])
    nc.compile()
    return nc


# --------------------------------------------------------------------------
# launch 3: per-core (b, node-quarter): inter-view MHA over V=4
# --------------------------------------------------------------------------
NQ = N // 4          # 2500 nodes per core
CH = 125             # chunk of nodes
NCH = NQ // CH       # 20


def build_launch3():
    hd = D // H      # 32
    nc = bacc.Bacc("TRN2", target_bir_lowering=False, debug=False,
                   num_devices=NCORES)
    xT4 = nc.dram_tensor("xT4", [V, P, NQ], FP32, kind="ExternalInput")
    wiT = nc.dram_tensor("wiT", [P, 3 * D], FP32, kind="ExternalInput")
    bi = nc.dram_tensor("bi", [1, 3 * D], FP32, kind="ExternalInput")
    woT = nc.dram_tensor("woT", [P, D], FP32, kind="ExternalInput")
    bo = nc.dram_tensor("bo", [1, D], FP32, kind="ExternalInput")
    bb = nc.dram_tensor("bb", [1, D], FP32, kind="ExternalInput")
    o_out = nc.dram_tensor("o", [V, NQ, D], FP32, kind="ExternalOutput")

    with tile.TileContext(nc) as tc:
        with tc.tile_pool(name="one", bufs=1) as one, \
             tc.tile_pool(name="sb", bufs=3) as sb, \
             tc.tile_pool(name="qkvp", bufs=6) as qkvp, \
             tc.tile_pool(name="ps", bufs=2, space="PSUM") as ps, \
             tc.tile_pool(name="ps2", bufs=2, space="PSUM") as ps2:
            identity = one.tile([P, P], FP32)
            make_identity(nc, identity[:])
            x_sb = one.tile([P, V * NQ], FP32)
            nc.sync.dma_start(x_sb[:].rearrange("d (v n) -> d v n", v=V),
                              xT4.ap().rearrange("v d n -> d v n"))
            wi_sb = one.tile([P, 3 * D], FP32)
            nc.sync.dma_start(wi_sb[:], wiT.ap()[:])
            wo_sb = one.tile([P, D], FP32)
            nc.sync.dma_start(wo_sb[:], woT.ap()[:])
            bi_row = one.tile([1, 3 * D], FP32)
            nc.sync.dma_start(bi_row[:], bi.ap()[:])
            bi_sb = one.tile([P, 3 * D], FP32)
            nc.gpsimd.partition_broadcast(bi_sb[:], bi_row[:])
            bo_row = one.tile([1, D], FP32)
            nc.sync.dma_start(bo_row[:], bo.ap()[:])
            bb_row = one.tile([1, D], FP32)
            nc.sync.dma_start(bb_row[:], bb.ap()[:])
            cb_row = one.tile([1, D], FP32)
            nc.vector.tensor_add(cb_row[:], bo_row[:], bb_row[:])
            cb_sb = one.tile([P, D], FP32)
            nc.gpsimd.partition_broadcast(cb_sb[:], cb_row[:])

            o_sb = one.tile([P, V * NCH * D], FP32)   # slot (q, c)

            for c in range(NCH):
                n0 = c * CH
                qkv = []
                for v in range(V):
                    q_ps = ps.tile([P, 3 * D], FP32, tag="qkv_ps")
                    nc.tensor.matmul(q_ps[:CH, :],
                                     x_sb[:, v * NQ + n0:v * NQ + n0 + CH],
                                     wi_sb[:], start=True, stop=True)
                    q_sb = qkvp.tile([P, 3 * D], FP32, tag="qkv")
                    nc.vector.tensor_add(q_sb[:CH, :], q_ps[:CH, :], bi_sb[:CH, :])
                    qkv.append(q_sb)
                # logits L layout [CH, (q, h, k)]
                L = sb.tile([P, V * H * V], FP32, tag="L")
                Lv = L[:].rearrange("p (q h k) -> p q h k", q=V, h=H)
                for q in range(V):
                    for k in range(V):
                        prod = sb.tile([P, D], FP32, tag="prod")
                        nc.vector.tensor_tensor(
                            out=prod[:CH, :], in0=qkv[q][:CH, 0:D],
                            in1=qkv[k][:CH, D:2 * D], op=mybir.AluOpType.mult)
                        nc.vector.tensor_reduce(
                            out=Lv[:CH, q, :, k],
                            in_=prod[:CH, :].rearrange("p (h f) -> p h f", h=H),
                            axis=mybir.AxisListType.X, op=mybir.AluOpType.add)
                # softmax over k
                M = sb.tile([P, V * H], FP32, tag="M")
                nc.vector.tensor_reduce(
                    out=M[:CH, :],
                    in_=L[:CH, :].rearrange("p (a k) -> p a k", k=V),
                    axis=mybir.AxisListType.X, op=mybir.AluOpType.max)
                Dm = sb.tile([P, V * H * V], FP32, tag="Dm")
                nc.vector.tensor_tensor(
                    out=Dm[:CH, :].rearrange("p (a k) -> p a k", k=V),
                    in0=L[:CH, :].rearrange("p (a k) -> p a k", k=V),
                    in1=M[:CH, :, None].to_broadcast([CH, V * H, V]),
                    op=mybir.AluOpType.subtract)
                Ex = sb.tile([P, V * H * V], FP32, tag="Ex")
                nc.scalar.activation(Ex[:CH, :], Dm[:CH, :],
                                     mybir.ActivationFunctionType.Exp,
                                     scale=1.0 / math.sqrt(hd))
                Ssum = sb.tile([P, V * H], FP32, tag="Ssum")
                nc.vector.tensor_reduce(
                    out=Ssum[:CH, :],
                    in_=Ex[:CH, :].rearrange("p (a k) -> p a k", k=V),
                    axis=mybir.AxisListType.X, op=mybir.AluOpType.add)
                R = sb.tile([P, V * H], FP32, tag="R")
                nc.vector.reciprocal(R[:CH, :], Ssum[:CH, :])
                A = sb.tile([P, V * H * V], FP32, tag="A")
                nc.vector.tensor_tensor(
                    out=A[:CH, :].rearrange("p (a k) -> p a k", k=V),
                    in0=Ex[:CH, :].rearrange("p (a k) -> p a k", k=V),
                    in1=R[:CH, :, None].to_broadcast([CH, V * H, V]),
                    op=mybir.AluOpType.mult)
                Av = A[:].rearrange("p (q h k) -> p q h k", q=V, h=H)
                # O_q = sum_k A[:,q,:,k] (x) V_k ; then out-proj
                for q in range(V):
                    O = sb.tile([P, D], FP32, tag="O")
                    Ov = O[:].rearrange("p (h f) -> p h f", h=H)
                    for k in range(V):
                        a_b = Av[:CH, q, :, k][:, :, None].to_broadcast([CH, H, hd])
                        vv = qkv[k][:CH, 2 * D:3 * D].rearrange(
                            "p (h f) -> p h f", h=H)
                        if k == 0:
                            nc.vector.tensor_tensor(out=Ov[:CH], in0=vv, in1=a_b,
                                                    op=mybir.AluOpType.mult)
                        else:
                            tmp = sb.tile([P, D], FP32, tag="avtmp")
                            tv = tmp[:].rearrange("p (h f) -> p h f", h=H)
                            nc.vector.tensor_tensor(out=tv[:CH], in0=vv, in1=a_b,
                                                    op=mybir.AluOpType.mult)
                            nc.vector.tensor_tensor(out=Ov[:CH], in0=Ov[:CH],
                                                    in1=tv[:CH],
                                                    op=mybir.AluOpType.add)
                    ot_ps = ps2.tile([P, P], FP32, tag="ot")
                    nc.tensor.transpose(ot_ps[:, :CH], O[:CH, :],
                                        identity[:CH, :CH])
                    oT = sb.tile([P, P], FP32, tag="oTsb")
                    nc.vector.tensor_copy(oT[:, :CH], ot_ps[:, :CH])
                    f_ps = ps2.tile([P, D], FP32, tag="f")
                    nc.tensor.matmul(f_ps[:CH, :], oT[:, :CH], wo_sb[:],
                                     start=True, stop=True)
                    nc.vector.tensor_add(
                        o_sb[:CH, (q * NCH + c) * D:(q * NCH + c + 1) * D],
                        f_ps[:CH, :], cb_sb[:CH, :])

            nc.sync.dma_start(
                o_out.ap().rearrange("v (c p) d -> p v c d", p=CH),
                o_sb[:CH, :].rearrange("p (v c d) -> p v c d", v=V, c=NCH))
    nc.compile()
    return nc


# --------------------------------------------------------------------------
# host orchestration
# --------------------------------------------------------------------------
_cache = {}
RUN_KW = {}       # extra kwargs for run_bass_kernel_spmd (e.g. trace=True)
EXEC_TIMES = {}   # launch name -> exec_time_ns (when tracing)


def _get(name, builder, *args):
    if name not in _cache:
        _cache[name] = builder(*args)
    return _cache[name]


def kernel(x, W, att, in_proj_w, in_proj_b, out_proj_w, out_proj_b, bias,
           edge_index):
    x = np.asarray(x, np.float32)
    W = np.asarray(W, np.float32)
    att = np.asarray(att, np.float32)
    in_proj_w = np.asarray(in_proj_w, np.float32)
    in_proj_b = np.asarray(in_proj_b, np.float32)
    out_proj_w = np.asarray(out_proj_w, np.float32)
    out_proj_b = np.asarray(out_proj_b, np.float32)
    bias = np.asarray(bias, np.float32)

    plan_key = np.asarray(edge_index).tobytes()
    if ("plan", plan_key) not in _cache:
        _cache[("plan", plan_key)] = prep_edges(edge_index)
    plan = _cache[("plan", plan_key)]

    # ---- launch 1 ----
    nc1 = _get("l1", build_launch1)
    xf = x.reshape(NCORES, N, FIN)
    wT = np.ascontiguousarray(W.T)                       # [64, 128]
    att2T = np.zeros((P, 2), np.float32)
    att2T[:, 0] = att[0, :, :F].reshape(-1)              # (h,f) flat src
    att2T[:, 1] = att[0, :, F:].reshape(-1)
    indsrc = np.zeros((P, 8), np.float32)
    inddst = np.zeros((P, 8), np.float32)
    for h in range(H):
        indsrc[h * F:(h + 1) * F, h] = 1.0
        inddst[h * F:(h + 1) * F, 4 + h] = 1.0
    in1 = [{"xT": np.ascontiguousarray(xf[c].T), "wT": wT, "att2T": att2T,
            "indsrc": indsrc, "inddst": inddst} for c in range(NCORES)]
    print("L1 run...", flush=True)
    r1 = run_bass_kernel_spmd(nc1, in1, core_ids=list(range(NCORES)), **RUN_KW)
    EXEC_TIMES["launch1"] = r1.exec_time_ns

    # ---- launch 2 ----
    sT = np.stack([r1.results[c]["sT"] for c in range(NCORES)])  # [8c, 8s, N]
    s_all = np.ascontiguousarray(sT.transpose(2, 1, 0).reshape(N, 64))
    nc2 = _get(("l2", plan.key), build_launch2, plan)
    in2 = [{"hT": r1.results[c]["hT"], "s_all": s_all, "idx16": plan.idx16,
            "rel": plan.rel} for c in range(NCORES)]
    print("L2 run...", flush=True)
    r2 = run_bass_kernel_spmd(nc2, in2, core_ids=list(range(NCORES)), **RUN_KW)
    EXEC_TIMES["launch2"] = r2.exec_time_ns

    # ---- launch 3 ----
    nc3 = _get("l3", build_launch3)
    wiT = np.ascontiguousarray(in_proj_w.T)              # [128, 384]
    woT = np.ascontiguousarray(out_proj_w.T)             # [128, 128]
    bi = in_proj_b.reshape(1, 3 * D)
    bo = out_proj_b.reshape(1, D)
    bb = bias.reshape(1, D)
    gatT = [r2.results[c]["gatT"] for c in range(NCORES)]  # each [128, N]
    in3 = []
    for c in range(NCORES):
        b, q = divmod(c, 4)
        xT4 = np.stack([gatT[b * V + v][:, q * NQ:(q + 1) * NQ]
                        for v in range(V)])              # [4, 128, 2500]
        in3.append({"xT4": np.ascontiguousarray(xT4), "wiT": wiT, "bi": bi,
                    "woT": woT, "bo": bo, "bb": bb})
    print("L3 run...", flush=True)
    r3 = run_bass_kernel_spmd(nc3, in3, core_ids=list(range(NCORES)), **RUN_KW)
    EXEC_TIMES["launch3"] = r3.exec_time_ns

    out = np.empty((B, V, N, D), np.float32)
    for c in range(NCORES):
        b, q = divmod(c, 4)
        out[b, :, q * NQ:(q + 1) * NQ, :] = r3.results[c]["o"]
    return out


# revision 15
# speedup vs baseline: 3.0451x; 3.0451x over previous
"""Trainium2 Bass kernel for nn_GATv2Layer4View (GAT message passing + inter-view MHA).

Self-contained: kernel(**inputs) -> np.ndarray [2, 4, 10000, 128] float32.

Math (faithful to reference):
  scores[e,h] = mean_bv(s_src[bv, src[e], h] + s_dst[bv, dst[e], h])   (node-separable)
  w = softmax(scores, axis=0) over ALL edges per head
    = ea[src[e],h] * eb[dst[e],h] / Z[h],  ea = exp(ms_src), eb = exp(ms_dst),
      Z = sum_e ea[src[e]] * eb[dst[e]]
  gat[bv,d,:] = (eb[d]/Z) (*) sum_{e: dst=d} (ea[src[e]] (*) h[bv, src[e]])
  -> pure unweighted gather + scatter-add of table rows; eb applied at the end;
     1/Z[h] folded into the MHA in_proj weight rows (launch 3).

Launch 1 (node-sharded, 1280 nodes/core): h for all 8 (b,v), per-node score
  means -> ea/eb, and the packed gather-table rows
  [ea*h_bv0 .. ea*h_bv7 (8*128 bf16) | ea (4) | pad] = 1152 bf16 = 2304 B.
Launch 2 (dst-node-range sharded): dma_gather of its ~24k edges' mega-rows,
  one-hot scatter matmul into PSUM (8 batches + ea column share one one-hot),
  *eb finalize (unnormalized), partial-Z output.
Launch 3 ((b, node-quarter) sharded): inter-view MHA over V=4, bf16 compute,
  with sum(Z partials) -> 1/Z scaling folded into in_proj_w rows.
"""

import math
import numpy as np
import ml_dtypes

import concourse.bass as bass
import concourse.bacc as bacc
import concourse.mybir as mybir
import concourse.tile as tile
import concourse.bass_isa as bass_isa
from concourse.bass_utils import run_bass_kernel_spmd
from concourse.masks import make_identity

P = 128
NCORES = 8
B, V, N, FIN = 2, 4, 10000, 64
H, F = 4, 32
D = H * F                      # 128
E_RAW = 160000
NEG_SLOPE = 0.2

NPC = 1280                     # nodes per core (node-sharded launches 1/2)
TPC = NPC // P                 # 10 tiles per core
TBL_COLS = 1152                # bf16 cols: 8*128 h' + 4 ea + 124 pad = 2304 B
EA_COL = 8 * D                 # 1024
GATHER_GROUP = 8               # chunks per dma_gather (1024 rows)

NQ = N // 4                    # 2500 nodes per core in launch 3
CH = 125
NCH = NQ // CH                 # 20

FP32 = mybir.dt.float32
BF16 = mybir.dt.bfloat16
I16 = mybir.dt.int16
I32 = mybir.dt.int32

BF = ml_dtypes.bfloat16

RUN_KW = {}
EXEC_TIMES = {}


# --------------------------------------------------------------------------
# host-side edge preprocessing (per-core dst ranges, uniform chunk structure)
# --------------------------------------------------------------------------
class EdgePlan:
    pass


def prep_edges(edge_index: np.ndarray) -> EdgePlan:
    ei = np.asarray(edge_index)
    src = np.concatenate([ei[0].astype(np.int64), np.arange(N)])
    dst = np.concatenate([ei[1].astype(np.int64), np.arange(N)])
    order = np.argsort(dst, kind="stable")
    ss, ds = src[order], dst[order]

    n_tiles_total = NCORES * TPC  # 80 tile slots (the last ones may be empty)
    bounds = np.searchsorted(ds, np.minimum(np.arange(n_tiles_total + 1) * P, N))
    counts = np.diff(bounds)
    cmax = int(math.ceil(counts.max() / P))

    idx_all = np.full((NCORES, TPC * cmax * P), N, np.int64)   # pad -> zero row
    rel_all = np.full((NCORES, TPC * cmax * P), 200.0, np.float32)
    for c in range(NCORES):
        for t in range(TPC):
            g = c * TPC + t
            k = bounds[g + 1] - bounds[g]
            o = t * cmax * P
            idx_all[c, o:o + k] = ss[bounds[g]:bounds[g + 1]]
            rel_all[c, o:o + k] = ds[bounds[g]:bounds[g + 1]] - g * P
    plan = EdgePlan()
    plan.cmax = cmax
    plan.idx16 = [np.ascontiguousarray(idx_all[c].astype(np.int16)
                                       .reshape(-1, 16).T) for c in range(NCORES)]
    plan.rel = [np.ascontiguousarray(rel_all[c].reshape(-1, P).T.astype(np.float32))
                for c in range(NCORES)]
    return plan


# --------------------------------------------------------------------------
# launch 1: node-sharded. h for all 8 bv + score means + ea/eb + table rows
# --------------------------------------------------------------------------
def build_launch1():
    nc = bacc.Bacc("TRN2", target_bir_lowering=False, debug=False,
                   num_devices=NCORES)
    xT = nc.dram_tensor("xT", [FIN, NCORES * NPC], BF16, kind="ExternalInput")
    wT = nc.dram_tensor("wT", [FIN, D], BF16, kind="ExternalInput")
    att2T = nc.dram_tensor("att2T", [P, 2], FP32, kind="ExternalInput")
    indsrc = nc.dram_tensor("indsrc", [P, 8], BF16, kind="ExternalInput")
    inddst = nc.dram_tensor("inddst", [P, 8], BF16, kind="ExternalInput")
    rows_out = nc.dram_tensor("rows", [NPC, TBL_COLS], BF16, kind="ExternalOutput")
    ee_out = nc.dram_tensor("eeT", [8, NPC], FP32, kind="ExternalOutput")

    with tile.TileContext(nc) as tc:
        with tc.tile_pool(name="one", bufs=1) as one, \
             tc.tile_pool(name="sb", bufs=3) as sb, \
             tc.tile_pool(name="hb", bufs=16) as hb, \
             tc.tile_pool(name="pk", bufs=2) as pk, \
             tc.tile_pool(name="psA", bufs=2, space="PSUM") as psA, \
             tc.tile_pool(name="psB", bufs=2, space="PSUM") as psB, \
             tc.tile_pool(name="psS", bufs=2, space="PSUM") as psS:
            identity = one.tile([P, P], BF16)
            make_identity(nc, identity[:])
            idf32 = one.tile([8, 8], FP32)
            make_identity(nc, idf32[:])
            xT_sb = one.tile([FIN, NCORES * NPC], BF16)
            nc.sync.dma_start(xT_sb[:], xT.ap()[:])
            wT_sb = one.tile([FIN, D], BF16)
            nc.sync.dma_start(wT_sb[:], wT.ap()[:])
            att_sb = one.tile([P, 2], FP32)
            nc.sync.dma_start(att_sb[:], att2T.ap()[:])
            ind_sb = one.tile([P, 16], BF16)
            nc.sync.dma_start(ind_sb[:, 0:8], indsrc.ap()[:])
            nc.sync.dma_start(ind_sb[:, 8:16], inddst.ap()[:])
            ee_sb = one.tile([8, NPC], FP32)

            for t in range(TPC):
                n0 = t * P
                s_ps = psS.tile([8, P], FP32, tag="s")
                hn_list = []
                for bv in range(8):
                    hT_ps = psA.tile([P, P], FP32, tag="hT")
                    nc.tensor.matmul(hT_ps[:], wT_sb[:],
                                     xT_sb[:, bv * NPC + n0:bv * NPC + n0 + P],
                                     start=True, stop=True)
                    t1 = sb.tile([P, P], FP32, tag="t1")
                    nc.scalar.mul(t1[:], hT_ps[:], NEG_SLOPE)
                    hl = sb.tile([P, P], FP32, tag="hl")
                    nc.vector.tensor_tensor(out=hl[:], in0=hT_ps[:], in1=t1[:],
                                            op=mybir.AluOpType.max)
                    psrc = sb.tile([P, P], BF16, tag="psrc")
                    nc.vector.tensor_scalar_mul(psrc[:], hl[:], att_sb[:, 0:1])
                    pdst = sb.tile([P, P], BF16, tag="pdst")
                    nc.vector.tensor_scalar_mul(pdst[:], hl[:], att_sb[:, 1:2])
                    nc.tensor.matmul(s_ps[:], ind_sb[:, 0:8], psrc[:],
                                     start=(bv == 0), stop=False)
                    nc.tensor.matmul(s_ps[:], ind_sb[:, 8:16], pdst[:],
                                     start=False, stop=(bv == 7))
                    hn = hb.tile([P, P], BF16, tag="hn")
                    nc.scalar.copy(hn[:], hT_ps[:])
                    hn_list.append(hn)
                nc.scalar.activation(ee_sb[:, n0:n0 + P], s_ps[:],
                                     mybir.ActivationFunctionType.Exp,
                                     scale=1.0 / 8.0)
                ee_ps = psB.tile([P, 8], FP32, tag="eeT")
                nc.tensor.transpose(ee_ps[:, 0:8], ee_sb[:, n0:n0 + P],
                                    idf32[:])
                ea_nm = sb.tile([P, 4], FP32, tag="ea")
                nc.vector.tensor_copy(ea_nm[:], ee_ps[:, 0:4])
                packed = pk.tile([P, EA_COL + 4], BF16, tag="packed")
                for bv in range(8):
                    hT_node_ps = psB.tile([P, P], BF16, tag="hnode")
                    nc.tensor.transpose(hT_node_ps[:], hn_list[bv][:], identity[:])
                    nc.vector.tensor_tensor(
                        out=packed[:, bv * D:(bv + 1) * D].rearrange(
                            "p (h f) -> p h f", h=H),
                        in0=hT_node_ps[:].rearrange("p (h f) -> p h f", h=H),
                        in1=ea_nm[:, :, None].to_broadcast([P, H, F]),
                        op=mybir.AluOpType.mult)
                nc.vector.tensor_copy(packed[:, EA_COL:EA_COL + 4], ea_nm[:])
                nc.sync.dma_start(rows_out.ap()[n0:n0 + P, 0:EA_COL + 4],
                                  packed[:])
            nc.sync.dma_start(ee_out.ap()[:], ee_sb[:])
    nc.compile()
    return nc


# --------------------------------------------------------------------------
# launch 2: dst-range edge aggregation (all 8 bv at once)
# --------------------------------------------------------------------------
def build_launch2(cmax: int):
    n_chunks = TPC * cmax
    idx_cols = n_chunks * P // 16

    nc = bacc.Bacc("TRN2", target_bir_lowering=False, debug=False,
                   num_devices=NCORES)
    tbl_in = nc.dram_tensor("table", [N + 1, TBL_COLS], BF16, kind="ExternalInput")
    idx_in = nc.dram_tensor("idx16", [16, idx_cols], I16, kind="ExternalInput")
    rel_in = nc.dram_tensor("rel", [P, n_chunks], FP32, kind="ExternalInput")
    ee_in = nc.dram_tensor("eeT", [8, NPC], FP32, kind="ExternalInput")
    gat_out = nc.dram_tensor("gatT", [8, P, NPC], FP32, kind="ExternalOutput")
    z_out = nc.dram_tensor("zpart", [1, H], FP32, kind="ExternalOutput")

    groups = []
    c = 0
    while c < n_chunks:
        m = min(GATHER_GROUP, n_chunks - c)
        groups.append((c, m))
        c += m

    with tile.TileContext(nc) as tc:
        with tc.tile_pool(name="one", bufs=1) as one, \
             tc.tile_pool(name="sb", bufs=3) as sb, \
             tc.tile_pool(name="gp", bufs=3) as gp, \
             tc.tile_pool(name="ps", bufs=1, space="PSUM") as ps, \
             tc.tile_pool(name="acc", bufs=2, space="PSUM") as accp:
            identity = one.tile([P, P], FP32)
            make_identity(nc, identity[:])
            iota_i = one.tile([P, P], I32)
            nc.gpsimd.iota(iota_i[:], [[1, P]], channel_multiplier=0)
            iota_b = one.tile([P, P], BF16)
            nc.vector.tensor_copy(iota_b[:], iota_i[:])

            idx_sb = one.tile([P, idx_cols], I16)
            for r in range(8):
                nc.sync.dma_start(idx_sb[16 * r:16 * (r + 1), :], idx_in.ap()[:])
            rel_sb = one.tile([P, n_chunks], FP32)
            nc.sync.dma_start(rel_sb[:], rel_in.ap()[:])
            ee_sb = one.tile([8, NPC], FP32)
            nc.sync.dma_start(ee_sb[:], ee_in.ap()[:])

            gatT_sb = one.tile([P, 8 * NPC], FP32)   # [d, (bv, node)]
            zacc = one.tile([P, H], FP32)
            nc.vector.memset(zacc[:], 0.0)

            acc_ps = None
            eb_nm = None
            for (c0, m) in groups:
                g = gp.tile([P, GATHER_GROUP, TBL_COLS], BF16, tag="g")
                nc.gpsimd.dma_gather(
                    out_ap=g[:, :m, :],
                    in_ap=tbl_in.ap()[:],
                    idxs_ap=idx_sb[:, c0 * 8:(c0 + m) * 8],
                    num_idxs=m * P,
                    num_idxs_reg=m * P,
                    elem_size=TBL_COLS,
                    single_packet=False,
                )
                for j in range(m):
                    ci = c0 + j
                    t, k = divmod(ci, cmax)
                    if k == 0:
                        acc_ps = accp.tile([P, EA_COL + 4], FP32, tag="acc")
                        eb_ps = ps.tile([P, 8], FP32, tag="ebT")
                        nc.tensor.transpose(
                            eb_ps[:, 0:8], ee_sb[:, t * P:(t + 1) * P],
                            identity[:8, :8])
                        eb_nm = sb.tile([P, 4], FP32, tag="eb")
                        nc.vector.tensor_copy(eb_nm[:], eb_ps[:, 4:8])
                    S = sb.tile([P, P], BF16, tag="S")
                    nc.vector.tensor_scalar(
                        out=S[:], in0=iota_b[:],
                        scalar1=rel_sb[:, ci:ci + 1], scalar2=None,
                        op0=mybir.AluOpType.is_equal)
                    # start=True zeroes the entire PSUM bank -> exactly one
                    # start per bank (bv0 -> bank0, bv4 -> bank1, ea -> bank2)
                    for bv in range(8):
                        nc.tensor.matmul(
                            acc_ps[:, bv * D:(bv + 1) * D], S[:],
                            g[:, j, bv * D:(bv + 1) * D],
                            start=(k == 0 and bv % 4 == 0),
                            stop=(k == cmax - 1),
                            skip_group_check=True)
                    nc.tensor.matmul(
                        acc_ps[:, EA_COL:EA_COL + 4], S[:],
                        g[:, j, EA_COL:EA_COL + 4],
                        start=(k == 0), stop=(k == cmax - 1),
                        skip_group_check=True)
                    if k == cmax - 1:
                        for bv in range(8):
                            om = sb.tile([P, D], FP32, tag="om")
                            nc.vector.tensor_tensor(
                                out=om[:].rearrange("p (h f) -> p h f", h=H),
                                in0=acc_ps[:, bv * D:(bv + 1) * D].rearrange(
                                    "p (h f) -> p h f", h=H),
                                in1=eb_nm[:, :, None].to_broadcast([P, H, F]),
                                op=mybir.AluOpType.mult)
                            o_ps = ps.tile([P, P], FP32, tag="oT")
                            nc.tensor.transpose(o_ps[:], om[:], identity[:])
                            nc.vector.tensor_copy(
                                gatT_sb[:, bv * NPC + t * P:bv * NPC + (t + 1) * P],
                                o_ps[:])
                        zp = sb.tile([P, H], FP32, tag="zp")
                        nc.vector.tensor_tensor(
                            out=zp[:], in0=acc_ps[:, EA_COL:EA_COL + 4],
                            in1=eb_nm[:], op=mybir.AluOpType.mult)
                        nc.vector.tensor_tensor(
                            out=zacc[:], in0=zacc[:], in1=zp[:],
                            op=mybir.AluOpType.add)

            zred = one.tile([P, H], FP32)
            nc.gpsimd.partition_all_reduce(zred[:], zacc[:], channels=P,
                                           reduce_op=bass_isa.ReduceOp.add)
            nc.sync.dma_start(z_out.ap()[:], zred[0:1, :])
            nc.sync.dma_start(
                gat_out.ap().rearrange("v d n -> d v n"),
                gatT_sb[:].rearrange("d (v n) -> d v n", v=8))
    nc.compile()
    return nc


# --------------------------------------------------------------------------
# launch 3: inter-view MHA (bf16), 1/Z folded into the x scaling
# --------------------------------------------------------------------------
def build_launch3():
    hd = D // H      # 32
    nc = bacc.Bacc("TRN2", target_bir_lowering=False, debug=False,
                   num_devices=NCORES)
    xT4 = nc.dram_tensor("xT4", [V, P, NQ], FP32, kind="ExternalInput")
    wiT = nc.dram_tensor("wiT", [P, 3 * D], FP32, kind="ExternalInput")
    bi = nc.dram_tensor("bi", [1, 3 * D], FP32, kind="ExternalInput")
    woT = nc.dram_tensor("woT", [P, D], BF16, kind="ExternalInput")
    bo = nc.dram_tensor("bo", [1, D], FP32, kind="ExternalInput")
    bb = nc.dram_tensor("bb", [1, D], FP32, kind="ExternalInput")
    zparts = nc.dram_tensor("zparts", [8, H], FP32, kind="ExternalInput")
    o_out = nc.dram_tensor("o", [V, NQ, D], FP32, kind="ExternalOutput")

    with tile.TileContext(nc) as tc:
        with tc.tile_pool(name="one", bufs=1) as one, \
             tc.tile_pool(name="sb", bufs=3) as sb, \
             tc.tile_pool(name="qkvp", bufs=6) as qkvp, \
             tc.tile_pool(name="ps", bufs=2, space="PSUM") as ps, \
             tc.tile_pool(name="ps2", bufs=2, space="PSUM") as ps2:
            identity = one.tile([P, P], BF16)
            make_identity(nc, identity[:])
            zp_sb = one.tile([8, H], FP32)
            nc.sync.dma_start(zp_sb[:], zparts.ap()[:])
            zsum = one.tile([8, H], FP32)
            nc.gpsimd.partition_all_reduce(zsum[:], zp_sb[:], channels=8,
                                           reduce_op=bass_isa.ReduceOp.add)
            rz = one.tile([1, H], FP32)
            nc.vector.reciprocal(rz[:], zsum[0:1, :])
            rzrow = one.tile([1, D], FP32)
            nc.vector.tensor_copy(rzrow[:].rearrange("p (h f) -> p h f", h=H),
                                  rz[:, :, None].to_broadcast([1, H, hd]))
            idf = one.tile([1, 1], FP32)
            nc.vector.memset(idf[:], 1.0)
            rz_ps = ps.tile([P, 1], FP32, tag="rzT")
            nc.tensor.transpose(rz_ps[:, 0:1], rzrow[:], idf[:])
            rzcol = one.tile([P, 1], FP32)
            nc.vector.tensor_copy(rzcol[:], rz_ps[:, 0:1])
            x_sb = one.tile([P, V * NQ], FP32)
            nc.sync.dma_start(x_sb[:].rearrange("d (v n) -> d v n", v=V),
                              xT4.ap().rearrange("v d n -> d v n"))
            xb_sb = one.tile([P, V * NQ], BF16)
            nc.vector.tensor_scalar_mul(xb_sb[:], x_sb[:], rzcol[:, 0:1])

            wi_f = one.tile([P, 3 * D], FP32)
            nc.sync.dma_start(wi_f[:], wiT.ap()[:])
            wi_sb = one.tile([P, 3 * D], BF16)
            nc.vector.tensor_copy(wi_sb[:], wi_f[:])
            wo_sb = one.tile([P, D], BF16)
            nc.sync.dma_start(wo_sb[:], woT.ap()[:])
            bi_row = one.tile([1, 3 * D], FP32)
            nc.sync.dma_start(bi_row[:], bi.ap()[:])
            bi_sb = one.tile([P, 3 * D], FP32)
            nc.gpsimd.partition_broadcast(bi_sb[:], bi_row[:])
            bo_row = one.tile([1, D], FP32)
            nc.sync.dma_start(bo_row[:], bo.ap()[:])
            bb_row = one.tile([1, D], FP32)
            nc.sync.dma_start(bb_row[:], bb.ap()[:])
            cb_row = one.tile([1, D], FP32)
            nc.vector.tensor_add(cb_row[:], bo_row[:], bb_row[:])
            cb_sb = one.tile([P, D], FP32)
            nc.gpsimd.partition_broadcast(cb_sb[:], cb_row[:])

            o_sb = one.tile([P, V * NCH * D], FP32)   # slot (q, c)

            for c in range(NCH):
                n0 = c * CH
                qkv = []
                for v in range(V):
                    q_ps = ps.tile([P, 3 * D], FP32, tag="qkv_ps")
                    nc.tensor.matmul(q_ps[:CH, :],
                                     xb_sb[:, v * NQ + n0:v * NQ + n0 + CH],
                                     wi_sb[:], start=True, stop=True)
                    q_sb = qkvp.tile([P, 3 * D], BF16, tag="qkv")
                    nc.vector.tensor_add(q_sb[:CH, :], q_ps[:CH, :], bi_sb[:CH, :])
                    qkv.append(q_sb)
                L = sb.tile([P, V * H * V], FP32, tag="L")
                Lv = L[:].rearrange("p (q h k) -> p q h k", q=V, h=H)
                for q in range(V):
                    for k in range(V):
                        prod = sb.tile([P, D], BF16, tag="prod")
                        nc.vector.tensor_tensor(
                            out=prod[:CH, :], in0=qkv[q][:CH, 0:D],
                            in1=qkv[k][:CH, D:2 * D], op=mybir.AluOpType.mult)
                        nc.vector.tensor_reduce(
                            out=Lv[:CH, q, :, k],
                            in_=prod[:CH, :].rearrange("p (h f) -> p h f", h=H),
                            axis=mybir.AxisListType.X, op=mybir.AluOpType.add)
                M = sb.tile([P, V * H], FP32, tag="M")
                nc.vector.tensor_reduce(
                    out=M[:CH, :],
                    in_=L[:CH, :].rearrange("p (a k) -> p a k", k=V),
                    axis=mybir.AxisListType.X, op=mybir.AluOpType.max)
                Dm = sb.tile([P, V * H * V], FP32, tag="Dm")
                nc.vector.tensor_tensor(
                    out=Dm[:CH, :].rearrange("p (a k) -> p a k", k=V),
                    in0=L[:CH, :].rearrange("p (a k) -> p a k", k=V),
                    in1=M[:CH, :, None].to_broadcast([CH, V * H, V]),
                    op=mybir.AluOpType.subtract)
                Ex = sb.tile([P, V * H * V], FP32, tag="Ex")
                nc.scalar.activation(Ex[:CH, :], Dm[:CH, :],
                                     mybir.ActivationFunctionType.Exp,
                                     scale=1.0 / math.sqrt(hd))
                Ssum = sb.tile([P, V * H], FP32, tag="Ssum")
                nc.vector.tensor_reduce(
                    out=Ssum[:CH, :],
                    in_=Ex[:CH, :].rearrange("p (a k) -> p a k", k=V),
                    axis=mybir.AxisListType.X, op=mybir.AluOpType.add)
                R = sb.tile([P, V * H], FP32, tag="R")
                nc.vector.reciprocal(R[:CH, :], Ssum[:CH, :])
                A = sb.tile([P, V * H * V], BF16, tag="A")
                nc.vector.tensor_tensor(
                    out=A[:CH, :].rearrange("p (a k) -> p a k", k=V),
                    in0=Ex[:CH, :].rearrange("p (a k) -> p a k", k=V),
                    in1=R[:CH, :, None].to_broadcast([CH, V * H, V]),
                    op=mybir.AluOpType.mult)
                Av = A[:].rearrange("p (q h k) -> p q h k", q=V, h=H)
                for q in range(V):
                    O = sb.tile([P, D], BF16, tag="O")
                    Ov = O[:].rearrange("p (h f) -> p h f", h=H)
                    for k in range(V):
                        a_b = Av[:CH, q, :, k][:, :, None].to_broadcast([CH, H, hd])
                        vv = qkv[k][:CH, 2 * D:3 * D].rearrange(
                            "p (h f) -> p h f", h=H)
                        if k == 0:
                            nc.vector.tensor_tensor(out=Ov[:CH], in0=vv, in1=a_b,
                                                    op=mybir.AluOpType.mult)
                        else:
                            tmp = sb.tile([P, D], BF16, tag="avtmp")
                            tv = tmp[:].rearrange("p (h f) -> p h f", h=H)
                            nc.vector.tensor_tensor(out=tv[:CH], in0=vv, in1=a_b,
                                                    op=mybir.AluOpType.mult)
                            nc.vector.tensor_tensor(out=Ov[:CH], in0=Ov[:CH],
                                                    in1=tv[:CH],
                                                    op=mybir.AluOpType.add)
                    ot_ps = ps2.tile([P, P], BF16, tag="ot")
                    nc.tensor.transpose(ot_ps[:, :CH], O[:CH, :],
                                        identity[:CH, :CH])
                    oT = sb.tile([P, P], BF16, tag="oTsb")
                    nc.vector.tensor_copy(oT[:, :CH], ot_ps[:, :CH])
                    f_ps = ps2.tile([P, D], FP32, tag="f")
                    nc.tensor.matmul(f_ps[:CH, :], oT[:, :CH], wo_sb[:],
                                     start=True, stop=True)
                    nc.vector.tensor_add(
                        o_sb[:CH, (q * NCH + c) * D:(q * NCH + c + 1) * D],
                        f_ps[:CH, :], cb_sb[:CH, :])

            nc.sync.dma_start(
                o_out.ap().rearrange("v (c p) d -> p v c d", p=CH),
                o_sb[:CH, :].rearrange("p (v c d) -> p v c d", v=V, c=NCH))
    nc.compile()
    return nc


# --------------------------------------------------------------------------
# host orchestration
# --------------------------------------------------------------------------
_cache = {}


def _get(name, builder, *args):
    if name not in _cache:
        _cache[name] = builder(*args)
    return _cache[name]


def kernel(x, W, att, in_proj_w, in_proj_b, out_proj_w, out_proj_b, bias,
           edge_index):
    x = np.asarray(x, np.float32)
    W = np.asarray(W, np.float32)
    att = np.asarray(att, np.float32)
    in_proj_w = np.asarray(in_proj_w, np.float32)
    in_proj_b = np.asarray(in_proj_b, np.float32)
    out_proj_w = np.asarray(out_proj_w, np.float32)
    out_proj_b = np.asarray(out_proj_b, np.float32)
    bias = np.asarray(bias, np.float32)

    plan_key = np.asarray(edge_index).tobytes()
    if ("plan", plan_key) not in _cache:
        _cache[("plan", plan_key)] = prep_edges(edge_index)
    plan = _cache[("plan", plan_key)]

    # ---- launch 1 ----
    nc1 = _get("l1", build_launch1)
    xf = x.reshape(NCORES, N, FIN)                        # [bv, n, fin]
    xpad = np.zeros((NCORES, NCORES * NPC, FIN), BF)
    xpad[:, :N, :] = xf.astype(BF)
    wT = np.ascontiguousarray(W.T.astype(BF))             # [64, 128]
    att2T = np.zeros((P, 2), np.float32)
    att2T[:, 0] = att[0, :, :F].reshape(-1)
    att2T[:, 1] = att[0, :, F:].reshape(-1)
    indsrc = np.zeros((P, 8), BF)
    inddst = np.zeros((P, 8), BF)
    for h in range(H):
        indsrc[h * F:(h + 1) * F, h] = 1.0
        inddst[h * F:(h + 1) * F, 4 + h] = 1.0
    in1 = []
    for c in range(NCORES):
        r0 = c * NPC
        sl = xpad[:, r0:r0 + NPC, :]                      # [8, NPC, 64]
        xT_c = np.ascontiguousarray(sl.transpose(2, 0, 1).reshape(FIN, -1))
        in1.append({"xT": xT_c, "wT": wT, "att2T": att2T,
                    "indsrc": indsrc, "inddst": inddst})
    r1 = run_bass_kernel_spmd(nc1, in1, core_ids=list(range(NCORES)), **RUN_KW)
    EXEC_TIMES["launch1"] = r1.exec_time_ns

    # ---- launch 2 ----
    rows = np.concatenate([r1.results[c]["rows"] for c in range(NCORES)])
    table = np.zeros((N + 1, TBL_COLS), BF)
    table[:N, :EA_COL + 4] = rows[:N, :EA_COL + 4]
    ee_full = np.concatenate([r1.results[c]["eeT"] for c in range(NCORES)],
                             axis=1)                      # [8, 10240]
    nc2 = _get(("l2", plan.cmax), build_launch2, plan.cmax)
    in2 = [{"table": table, "idx16": plan.idx16[c], "rel": plan.rel[c],
            "eeT": np.ascontiguousarray(ee_full[:, c * NPC:(c + 1) * NPC])}
           for c in range(NCORES)]
    r2 = run_bass_kernel_spmd(nc2, in2, core_ids=list(range(NCORES)), **RUN_KW)
    EXEC_TIMES["launch2"] = r2.exec_time_ns

    # ---- launch 3 ----
    nc3 = _get("l3", build_launch3)
    gatT = np.concatenate([r2.results[c]["gatT"] for c in range(NCORES)],
                          axis=2)                         # [8, 128, 10240]
    zparts = np.stack([r2.results[c]["zpart"][0] for c in range(NCORES)])
    wiT = np.ascontiguousarray(in_proj_w.T)               # [128, 384]
    woT = np.ascontiguousarray(out_proj_w.T.astype(BF))   # [128, 128]
    bi = np.ascontiguousarray(in_proj_b.reshape(1, 3 * D))
    bo = np.ascontiguousarray(out_proj_b.reshape(1, D))
    bb = np.ascontiguousarray(bias.reshape(1, D))
    in3 = []
    for c in range(NCORES):
        b, q = divmod(c, 4)
        xT4 = np.ascontiguousarray(
            gatT[b * V:(b + 1) * V, :, q * NQ:(q + 1) * NQ])  # [4, 128, 2500]
        in3.append({"xT4": xT4, "wiT": wiT, "bi": bi, "woT": woT,
                    "bo": bo, "bb": bb, "zparts": zparts})
    r3 = run_bass_kernel_spmd(nc3, in3, core_ids=list(range(NCORES)), **RUN_KW)
    EXEC_TIMES["launch3"] = r3.exec_time_ns

    out = np.empty((B, V, N, D), np.float32)
    for c in range(NCORES):
        b, q = divmod(c, 4)
        out[b, :, q * NQ:(q + 1) * NQ, :] = r3.results[c]["o"]
    return out


# revision 18
# speedup vs baseline: 3.3287x; 1.0931x over previous
"""Trainium2 Bass kernel for nn_GATv2Layer4View (GAT message passing + inter-view MHA).

Self-contained: kernel(**inputs) -> np.ndarray [2, 4, 10000, 128] float32.

Math (faithful to reference):
  scores[e,h] = mean_bv(s_src[bv, src[e], h] + s_dst[bv, dst[e], h])   (node-separable)
  w = softmax(scores, axis=0) over ALL edges per head
    = ea[src[e],h] * eb[dst[e],h] / Z[h],  ea = exp(ms_src), eb = exp(ms_dst),
      Z = sum_e ea[src[e]] * eb[dst[e]]
  gat[bv,d,:] = (eb[d]/Z) (*) sum_{e: dst=d} (ea[src[e]] (*) h[bv, src[e]])
  -> pure unweighted gather + scatter-add of table rows; eb applied at the end;
     1/Z[h] folded into the MHA in_proj weight rows (launch 3).

Launch 1 (node-sharded, 1280 nodes/core): h for all 8 (b,v), per-node score
  means -> ea/eb, and the packed gather-table rows
  [ea*h_bv0 .. ea*h_bv7 (8*128 bf16) | ea (4) | pad] = 1152 bf16 = 2304 B.
Launch 2 (dst-node-range sharded): dma_gather of its ~24k edges' mega-rows,
  one-hot scatter matmul into PSUM (8 batches + ea column share one one-hot),
  *eb finalize (unnormalized), partial-Z output.
Launch 3 ((b, node-quarter) sharded): inter-view MHA over V=4, bf16 compute,
  with sum(Z partials) -> 1/Z scaling folded into in_proj_w rows.
"""

import math
import numpy as np
import ml_dtypes

import concourse.bass as bass
import concourse.bacc as bacc
import concourse.mybir as mybir
import concourse.tile as tile
import concourse.bass_isa as bass_isa
from concourse.bass_utils import run_bass_kernel_spmd
from concourse.masks import make_identity

P = 128
NCORES = 8
B, V, N, FIN = 2, 4, 10000, 64
H, F = 4, 32
D = H * F                      # 128
E_RAW = 160000
NEG_SLOPE = 0.2

NPC = 1280                     # nodes per core (node-sharded launches 1/2)
TPC = NPC // P                 # 10 tiles per core
TBL_COLS = 1152                # bf16 cols: 8*128 h' + 4 ea + 124 pad = 2304 B
EA_COL = 8 * D                 # 1024
GATHER_GROUP = 8               # chunks per dma_gather (1024 rows)

NQ = N // 4                    # 2500 nodes per core in launch 3
CH = 125
NCH = NQ // CH                 # 20

FP32 = mybir.dt.float32
BF16 = mybir.dt.bfloat16
I16 = mybir.dt.int16
I32 = mybir.dt.int32

BF = ml_dtypes.bfloat16

RUN_KW = {}
EXEC_TIMES = {}


# --------------------------------------------------------------------------
# host-side edge preprocessing (per-core dst ranges, uniform chunk structure)
# --------------------------------------------------------------------------
class EdgePlan:
    pass


def prep_edges(edge_index: np.ndarray) -> EdgePlan:
    ei = np.asarray(edge_index)
    src = np.concatenate([ei[0].astype(np.int64), np.arange(N)])
    dst = np.concatenate([ei[1].astype(np.int64), np.arange(N)])
    order = np.argsort(dst, kind="stable")
    ss, ds = src[order], dst[order]

    n_tiles_total = NCORES * TPC  # 80 tile slots (the last ones may be empty)
    bounds = np.searchsorted(ds, np.minimum(np.arange(n_tiles_total + 1) * P, N))
    counts = np.diff(bounds)
    cmax = int(math.ceil(counts.max() / P))

    idx_all = np.full((NCORES, TPC * cmax * P), N, np.int64)   # pad -> zero row
    rel_all = np.full((NCORES, TPC * cmax * P), 200.0, np.float32)
    for c in range(NCORES):
        for t in range(TPC):
            g = c * TPC + t
            k = bounds[g + 1] - bounds[g]
            o = t * cmax * P
            idx_all[c, o:o + k] = ss[bounds[g]:bounds[g + 1]]
            rel_all[c, o:o + k] = ds[bounds[g]:bounds[g + 1]] - g * P
    plan = EdgePlan()
    plan.cmax = cmax
    plan.idx16 = [np.ascontiguousarray(idx_all[c].astype(np.int16)
                                       .reshape(-1, 16).T) for c in range(NCORES)]
    plan.rel = [np.ascontiguousarray(rel_all[c].reshape(-1, P).T.astype(np.float32))
                for c in range(NCORES)]
    return plan


# --------------------------------------------------------------------------
# launch 1: node-sharded. h for all 8 bv + score means + ea/eb + table rows
# --------------------------------------------------------------------------
def build_launch1():
    nc = bacc.Bacc("TRN2", target_bir_lowering=False, debug=False,
                   num_devices=NCORES)
    xT = nc.dram_tensor("xT", [FIN, NCORES * NPC], BF16, kind="ExternalInput")
    wT = nc.dram_tensor("wT", [FIN, D], BF16, kind="ExternalInput")
    att2T = nc.dram_tensor("att2T", [P, 2], FP32, kind="ExternalInput")
    indsrc = nc.dram_tensor("indsrc", [P, 8], BF16, kind="ExternalInput")
    inddst = nc.dram_tensor("inddst", [P, 8], BF16, kind="ExternalInput")
    rows_out = nc.dram_tensor("rows", [NPC, TBL_COLS], BF16, kind="ExternalOutput")
    ee_out = nc.dram_tensor("eeT", [8, NPC], FP32, kind="ExternalOutput")

    with tile.TileContext(nc) as tc:
        with tc.tile_pool(name="one", bufs=1) as one, \
             tc.tile_pool(name="sb", bufs=3) as sb, \
             tc.tile_pool(name="hb", bufs=16) as hb, \
             tc.tile_pool(name="pk", bufs=2) as pk, \
             tc.tile_pool(name="psA", bufs=2, space="PSUM") as psA, \
             tc.tile_pool(name="psB", bufs=2, space="PSUM") as psB, \
             tc.tile_pool(name="psS", bufs=2, space="PSUM") as psS:
            identity = one.tile([P, P], BF16)
            make_identity(nc, identity[:])
            idf32 = one.tile([8, 8], FP32)
            make_identity(nc, idf32[:])
            xT_sb = one.tile([FIN, NCORES * NPC], BF16)
            nc.sync.dma_start(xT_sb[:], xT.ap()[:])
            wT_sb = one.tile([FIN, D], BF16)
            nc.sync.dma_start(wT_sb[:], wT.ap()[:])
            att_sb = one.tile([P, 2], FP32)
            nc.sync.dma_start(att_sb[:], att2T.ap()[:])
            ind_sb = one.tile([P, 16], BF16)
            nc.sync.dma_start(ind_sb[:, 0:8], indsrc.ap()[:])
            nc.sync.dma_start(ind_sb[:, 8:16], inddst.ap()[:])
            ee_sb = one.tile([8, NPC], FP32)

            for t in range(TPC):
                n0 = t * P
                s_ps = psS.tile([8, P], FP32, tag="s")
                hn_list = []
                for bv in range(8):
                    hT_ps = psA.tile([P, P], FP32, tag="hT")
                    nc.tensor.matmul(hT_ps[:], wT_sb[:],
                                     xT_sb[:, bv * NPC + n0:bv * NPC + n0 + P],
                                     start=True, stop=True)
                    t1 = sb.tile([P, P], FP32, tag="t1")
                    nc.scalar.mul(t1[:], hT_ps[:], NEG_SLOPE)
                    hl = sb.tile([P, P], FP32, tag="hl")
                    nc.vector.tensor_tensor(out=hl[:], in0=hT_ps[:], in1=t1[:],
                                            op=mybir.AluOpType.max)
                    psrc = sb.tile([P, P], BF16, tag="psrc")
                    nc.vector.tensor_scalar_mul(psrc[:], hl[:], att_sb[:, 0:1])
                    pdst = sb.tile([P, P], BF16, tag="pdst")
                    nc.vector.tensor_scalar_mul(pdst[:], hl[:], att_sb[:, 1:2])
                    nc.tensor.matmul(s_ps[:], ind_sb[:, 0:8], psrc[:],
                                     start=(bv == 0), stop=False)
                    nc.tensor.matmul(s_ps[:], ind_sb[:, 8:16], pdst[:],
                                     start=False, stop=(bv == 7))
                    hn = hb.tile([P, P], BF16, tag="hn")
                    nc.scalar.copy(hn[:], hT_ps[:])
                    hn_list.append(hn)
                nc.scalar.activation(ee_sb[:, n0:n0 + P], s_ps[:],
                                     mybir.ActivationFunctionType.Exp,
                                     scale=1.0 / 8.0)
                ee_ps = psB.tile([P, 8], FP32, tag="eeT")
                nc.tensor.transpose(ee_ps[:, 0:8], ee_sb[:, n0:n0 + P],
                                    idf32[:])
                ea_nm = sb.tile([P, 4], FP32, tag="ea")
                nc.vector.tensor_copy(ea_nm[:], ee_ps[:, 0:4])
                packed = pk.tile([P, EA_COL + 4], BF16, tag="packed")
                for bv in range(8):
                    hT_node_ps = psB.tile([P, P], BF16, tag="hnode")
                    nc.tensor.transpose(hT_node_ps[:], hn_list[bv][:], identity[:])
                    nc.vector.tensor_tensor(
                        out=packed[:, bv * D:(bv + 1) * D].rearrange(
                            "p (h f) -> p h f", h=H),
                        in0=hT_node_ps[:].rearrange("p (h f) -> p h f", h=H),
                        in1=ea_nm[:, :, None].to_broadcast([P, H, F]),
                        op=mybir.AluOpType.mult)
                nc.vector.tensor_copy(packed[:, EA_COL:EA_COL + 4], ea_nm[:])
                nc.sync.dma_start(rows_out.ap()[n0:n0 + P, 0:EA_COL + 4],
                                  packed[:])
            nc.sync.dma_start(ee_out.ap()[:], ee_sb[:])
    nc.compile()
    return nc


# --------------------------------------------------------------------------
# launch 2: dst-range edge aggregation (all 8 bv at once)
# --------------------------------------------------------------------------
def build_launch2(cmax: int):
    n_chunks = TPC * cmax
    idx_cols = n_chunks * P // 16

    nc = bacc.Bacc("TRN2", target_bir_lowering=False, debug=False,
                   num_devices=NCORES)
    tbl_in = nc.dram_tensor("table", [N + 1, TBL_COLS], BF16, kind="ExternalInput")
    idx_in = nc.dram_tensor("idx16", [16, idx_cols], I16, kind="ExternalInput")
    rel_in = nc.dram_tensor("rel", [P, n_chunks], FP32, kind="ExternalInput")
    ee_in = nc.dram_tensor("eeT", [8, NPC], FP32, kind="ExternalInput")
    gat_out = nc.dram_tensor("gatT", [8, P, NPC], FP32, kind="ExternalOutput")
    z_out = nc.dram_tensor("zpart", [1, H], FP32, kind="ExternalOutput")

    groups = []
    c = 0
    while c < n_chunks:
        m = min(GATHER_GROUP, n_chunks - c)
        groups.append((c, m))
        c += m

    with tile.TileContext(nc) as tc:
        with tc.tile_pool(name="one", bufs=1) as one, \
             tc.tile_pool(name="sb", bufs=3) as sb, \
             tc.tile_pool(name="gp", bufs=3) as gp, \
             tc.tile_pool(name="ps", bufs=1, space="PSUM") as ps, \
             tc.tile_pool(name="acc", bufs=2, space="PSUM") as accp:
            identity = one.tile([P, P], FP32)
            make_identity(nc, identity[:])
            iota_i = one.tile([P, P], I32)
            nc.gpsimd.iota(iota_i[:], [[1, P]], channel_multiplier=0)
            iota_b = one.tile([P, P], BF16)
            nc.vector.tensor_copy(iota_b[:], iota_i[:])

            idx_sb = one.tile([P, idx_cols], I16)
            for r in range(8):
                nc.sync.dma_start(idx_sb[16 * r:16 * (r + 1), :], idx_in.ap()[:])
            rel_sb = one.tile([P, n_chunks], FP32)
            nc.sync.dma_start(rel_sb[:], rel_in.ap()[:])
            ee_sb = one.tile([8, NPC], FP32)
            nc.sync.dma_start(ee_sb[:], ee_in.ap()[:])

            gatT_sb = one.tile([P, 8 * NPC], FP32)   # [d, (bv, node)]
            zacc = one.tile([P, H], FP32)
            nc.vector.memset(zacc[:], 0.0)

            # precompute all one-hots + per-tile eb before the gather phase
            # (during gathers, SWDGE descriptor traffic slows DVE 5-9x)
            S_all = one.tile([P, n_chunks * P], BF16)
            for ci in range(n_chunks):
                nc.vector.tensor_scalar(
                    out=S_all[:, ci * P:(ci + 1) * P], in0=iota_b[:],
                    scalar1=rel_sb[:, ci:ci + 1], scalar2=None,
                    op0=mybir.AluOpType.is_equal)
            eb_all = one.tile([P, TPC * 4], FP32)
            for t in range(TPC):
                eb_ps = ps.tile([P, 8], FP32, tag="ebT")
                nc.tensor.transpose(eb_ps[:, 0:8], ee_sb[:, t * P:(t + 1) * P],
                                    identity[:8, :8])
                nc.vector.tensor_copy(eb_all[:, t * 4:(t + 1) * 4],
                                      eb_ps[:, 4:8])

            acc_ps = None
            for (c0, m) in groups:
                g = gp.tile([P, GATHER_GROUP, TBL_COLS], BF16, tag="g")
                nc.gpsimd.dma_gather(
                    out_ap=g[:, :m, :],
                    in_ap=tbl_in.ap()[:],
                    idxs_ap=idx_sb[:, c0 * 8:(c0 + m) * 8],
                    num_idxs=m * P,
                    num_idxs_reg=m * P,
                    elem_size=TBL_COLS,
                    single_packet=False,
                )
                for j in range(m):
                    ci = c0 + j
                    t, k = divmod(ci, cmax)
                    if k == 0:
                        acc_ps = accp.tile([P, EA_COL + 4], FP32, tag="acc")
                    S = S_all[:, ci * P:(ci + 1) * P]
                    # start=True zeroes the entire PSUM bank -> exactly one
                    # start per bank (one N=512 matmul per bank + ea)
                    for half in range(2):
                        nc.tensor.matmul(
                            acc_ps[:, half * 512:(half + 1) * 512], S,
                            g[:, j, half * 512:(half + 1) * 512],
                            start=(k == 0), stop=(k == cmax - 1),
                            skip_group_check=True)
                    nc.tensor.matmul(
                        acc_ps[:, EA_COL:EA_COL + 4], S,
                        g[:, j, EA_COL:EA_COL + 4],
                        start=(k == 0), stop=(k == cmax - 1),
                        skip_group_check=True)
                    if k == cmax - 1:
                        eb_nm = eb_all[:, t * 4:(t + 1) * 4]
                        for bv in range(8):
                            om = sb.tile([P, D], FP32, tag="om")
                            nc.vector.tensor_tensor(
                                out=om[:].rearrange("p (h f) -> p h f", h=H),
                                in0=acc_ps[:, bv * D:(bv + 1) * D].rearrange(
                                    "p (h f) -> p h f", h=H),
                                in1=eb_nm[:, :, None].to_broadcast([P, H, F]),
                                op=mybir.AluOpType.mult)
                            o_ps = ps.tile([P, P], FP32, tag="oT")
                            nc.tensor.transpose(o_ps[:], om[:], identity[:])
                            nc.vector.tensor_copy(
                                gatT_sb[:, bv * NPC + t * P:bv * NPC + (t + 1) * P],
                                o_ps[:])
                        zp = sb.tile([P, H], FP32, tag="zp")
                        nc.vector.tensor_tensor(
                            out=zp[:], in0=acc_ps[:, EA_COL:EA_COL + 4],
                            in1=eb_nm[:], op=mybir.AluOpType.mult)
                        nc.vector.tensor_tensor(
                            out=zacc[:], in0=zacc[:], in1=zp[:],
                            op=mybir.AluOpType.add)

            zred = one.tile([P, H], FP32)
            nc.gpsimd.partition_all_reduce(zred[:], zacc[:], channels=P,
                                           reduce_op=bass_isa.ReduceOp.add)
            nc.sync.dma_start(z_out.ap()[:], zred[0:1, :])
            nc.sync.dma_start(
                gat_out.ap().rearrange("v d n -> d v n"),
                gatT_sb[:].rearrange("d (v n) -> d v n", v=8))
    nc.compile()
    return nc


# --------------------------------------------------------------------------
# launch 3: inter-view MHA (bf16), 1/Z folded into the x scaling
# --------------------------------------------------------------------------
def build_launch3():
    hd = D // H      # 32
    nc = bacc.Bacc("TRN2", target_bir_lowering=False, debug=False,
                   num_devices=NCORES)
    xT4 = nc.dram_tensor("xT4", [V, P, NQ], FP32, kind="ExternalInput")
    wiT = nc.dram_tensor("wiT", [P, 3 * D], FP32, kind="ExternalInput")
    bi = nc.dram_tensor("bi", [1, 3 * D], FP32, kind="ExternalInput")
    woT = nc.dram_tensor("woT", [P, D], BF16, kind="ExternalInput")
    bo = nc.dram_tensor("bo", [1, D], FP32, kind="ExternalInput")
    bb = nc.dram_tensor("bb", [1, D], FP32, kind="ExternalInput")
    zparts = nc.dram_tensor("zparts", [8, H], FP32, kind="ExternalInput")
    o_out = nc.dram_tensor("o", [V, NQ, D], FP32, kind="ExternalOutput")

    with tile.TileContext(nc) as tc:
        with tc.tile_pool(name="one", bufs=1) as one, \
             tc.tile_pool(name="sb", bufs=3) as sb, \
             tc.tile_pool(name="qkvp", bufs=6) as qkvp, \
             tc.tile_pool(name="ps", bufs=2, space="PSUM") as ps, \
             tc.tile_pool(name="ps2", bufs=2, space="PSUM") as ps2:
            identity = one.tile([P, P], BF16)
            make_identity(nc, identity[:])
            zp_sb = one.tile([8, H], FP32)
            nc.sync.dma_start(zp_sb[:], zparts.ap()[:])
            zsum = one.tile([8, H], FP32)
            nc.gpsimd.partition_all_reduce(zsum[:], zp_sb[:], channels=8,
                                           reduce_op=bass_isa.ReduceOp.add)
            rz = one.tile([1, H], FP32)
            nc.vector.reciprocal(rz[:], zsum[0:1, :])
            rzrow = one.tile([1, D], FP32)
            nc.vector.tensor_copy(rzrow[:].rearrange("p (h f) -> p h f", h=H),
                                  rz[:, :, None].to_broadcast([1, H, hd]))
            idf = one.tile([1, 1], FP32)
            nc.vector.memset(idf[:], 1.0)
            rz_ps = ps.tile([P, 1], FP32, tag="rzT")
            nc.tensor.transpose(rz_ps[:, 0:1], rzrow[:], idf[:])
            rzcol = one.tile([P, 1], FP32)
            nc.vector.tensor_copy(rzcol[:], rz_ps[:, 0:1])
            x_sb = one.tile([P, V * NQ], FP32)
            nc.sync.dma_start(x_sb[:].rearrange("d (v n) -> d v n", v=V),
                              xT4.ap().rearrange("v d n -> d v n"))
            xb_sb = one.tile([P, V * NQ], BF16)
            nc.vector.tensor_scalar_mul(xb_sb[:], x_sb[:], rzcol[:, 0:1])

            wi_f = one.tile([P, 3 * D], FP32)
            nc.sync.dma_start(wi_f[:], wiT.ap()[:])
            wi_sb = one.tile([P, 3 * D], BF16)
            nc.vector.tensor_copy(wi_sb[:], wi_f[:])
            wo_sb = one.tile([P, D], BF16)
            nc.sync.dma_start(wo_sb[:], woT.ap()[:])
            bi_row = one.tile([1, 3 * D], FP32)
            nc.sync.dma_start(bi_row[:], bi.ap()[:])
            bi_sb = one.tile([P, 3 * D], FP32)
            nc.gpsimd.partition_broadcast(bi_sb[:], bi_row[:])
            bo_row = one.tile([1, D], FP32)
            nc.sync.dma_start(bo_row[:], bo.ap()[:])
            bb_row = one.tile([1, D], FP32)
            nc.sync.dma_start(bb_row[:], bb.ap()[:])
            cb_row = one.tile([1, D], FP32)
            nc.vector.tensor_add(cb_row[:], bo_row[:], bb_row[:])
            cb_sb = one.tile([P, D], FP32)
            nc.gpsimd.partition_broadcast(cb_sb[:], cb_row[:])

            o_sb = one.tile([P, V * NCH * D], FP32)   # slot (q, c)

            for c in range(NCH):
                n0 = c * CH
                qkv = []
                for v in range(V):
                    q_ps = ps.tile([P, 3 * D], FP32, tag="qkv_ps")
                    nc.tensor.matmul(q_ps[:CH, :],
                                     xb_sb[:, v * NQ + n0:v * NQ + n0 + CH],
                                     wi_sb[:], start=True, stop=True)
                    q_sb = qkvp.tile([P, 3 * D], BF16, tag="qkv")
                    nc.vector.tensor_add(q_sb[:CH, :], q_ps[:CH, :], bi_sb[:CH, :])
                    qkv.append(q_sb)
                L = sb.tile([P, V * H * V], FP32, tag="L")
                Lv = L[:].rearrange("p (q h k) -> p q h k", q=V, h=H)
                for q in range(V):
                    for k in range(V):
                        prod = sb.tile([P, D], BF16, tag="prod")
                        nc.vector.tensor_tensor(
                            out=prod[:CH, :], in0=qkv[q][:CH, 0:D],
                            in1=qkv[k][:CH, D:2 * D], op=mybir.AluOpType.mult)
                        nc.vector.tensor_reduce(
                            out=Lv[:CH, q, :, k],
                            in_=prod[:CH, :].rearrange("p (h f) -> p h f", h=H),
                            axis=mybir.AxisListType.X, op=mybir.AluOpType.add)
                M = sb.tile([P, V * H], FP32, tag="M")
                nc.vector.tensor_reduce(
                    out=M[:CH, :],
                    in_=L[:CH, :].rearrange("p (a k) -> p a k", k=V),
                    axis=mybir.AxisListType.X, op=mybir.AluOpType.max)
                Dm = sb.tile([P, V * H * V], FP32, tag="Dm")
                nc.vector.tensor_tensor(
                    out=Dm[:CH, :].rearrange("p (a k) -> p a k", k=V),
                    in0=L[:CH, :].rearrange("p (a k) -> p a k", k=V),
                    in1=M[:CH, :, None].to_broadcast([CH, V * H, V]),
                    op=mybir.AluOpType.subtract)
                Ex = sb.tile([P, V * H * V], FP32, tag="Ex")
                nc.scalar.activation(Ex[:CH, :], Dm[:CH, :],
                                     mybir.ActivationFunctionType.Exp,
                                     scale=1.0 / math.sqrt(hd))
                Ssum = sb.tile([P, V * H], FP32, tag="Ssum")
                nc.vector.tensor_reduce(
                    out=Ssum[:CH, :],
                    in_=Ex[:CH, :].rearrange("p (a k) -> p a k", k=V),
                    axis=mybir.AxisListType.X, op=mybir.AluOpType.add)
                R = sb.tile([P, V * H], FP32, tag="R")
                nc.vector.reciprocal(R[:CH, :], Ssum[:CH, :])
                A = sb.tile([P, V * H * V], BF16, tag="A")
                nc.vector.tensor_tensor(
                    out=A[:CH, :].rearrange("p (a k) -> p a k", k=V),
                    in0=Ex[:CH, :].rearrange("p (a k) -> p a k", k=V),
                    in1=R[:CH, :, None].to_broadcast([CH, V * H, V]),
                    op=mybir.AluOpType.mult)
                Av = A[:].rearrange("p (q h k) -> p q h k", q=V, h=H)
                for q in range(V):
                    O = sb.tile([P, D], BF16, tag="O")
                    Ov = O[:].rearrange("p (h f) -> p h f", h=H)
                    for k in range(V):
                        a_b = Av[:CH, q, :, k][:, :, None].to_broadcast([CH, H, hd])
                        vv = qkv[k][:CH, 2 * D:3 * D].rearrange(
                            "p (h f) -> p h f", h=H)
                        if k == 0:
                            nc.vector.tensor_tensor(out=Ov[:CH], in0=vv, in1=a_b,
                                                    op=mybir.AluOpType.mult)
                        else:
                            tmp = sb.tile([P, D], BF16, tag="avtmp")
                            tv = tmp[:].rearrange("p (h f) -> p h f", h=H)
                            eng = nc.gpsimd if k != 1 else nc.vector
                            eng.tensor_tensor(out=tv[:CH], in0=vv, in1=a_b,
                                              op=mybir.AluOpType.mult)
                            nc.vector.tensor_tensor(out=Ov[:CH], in0=Ov[:CH],
                                                    in1=tv[:CH],
                                                    op=mybir.AluOpType.add)
                    ot_ps = ps2.tile([P, P], BF16, tag="ot")
                    nc.tensor.transpose(ot_ps[:, :CH], O[:CH, :],
                                        identity[:CH, :CH])
                    oT = sb.tile([P, P], BF16, tag="oTsb")
                    nc.scalar.copy(oT[:, :CH], ot_ps[:, :CH])
                    f_ps = ps2.tile([P, D], FP32, tag="f")
                    nc.tensor.matmul(f_ps[:CH, :], oT[:, :CH], wo_sb[:],
                                     start=True, stop=True)
                    nc.vector.tensor_add(
                        o_sb[:CH, (q * NCH + c) * D:(q * NCH + c + 1) * D],
                        f_ps[:CH, :], cb_sb[:CH, :])

            nc.sync.dma_start(
                o_out.ap().rearrange("v (c p) d -> p v c d", p=CH),
                o_sb[:CH, :].rearrange("p (v c d) -> p v c d", v=V, c=NCH))
    nc.compile()
    return nc


# --------------------------------------------------------------------------
# host orchestration
# --------------------------------------------------------------------------
_cache = {}


def _get(name, builder, *args):
    if name not in _cache:
        _cache[name] = builder(*args)
    return _cache[name]


def kernel(x, W, att, in_proj_w, in_proj_b, out_proj_w, out_proj_b, bias,
           edge_index):
    x = np.asarray(x, np.float32)
    W = np.asarray(W, np.float32)
    att = np.asarray(att, np.float32)
    in_proj_w = np.asarray(in_proj_w, np.float32)
    in_proj_b = np.asarray(in_proj_b, np.float32)
    out_proj_w = np.asarray(out_proj_w, np.float32)
    out_proj_b = np.asarray(out_proj_b, np.float32)
    bias = np.asarray(bias, np.float32)

    plan_key = np.asarray(edge_index).tobytes()
    if ("plan", plan_key) not in _cache:
        _cache[("plan", plan_key)] = prep_edges(edge_index)
    plan = _cache[("plan", plan_key)]

    # ---- launch 1 ----
    nc1 = _get("l1", build_launch1)
    xf = x.reshape(NCORES, N, FIN)                        # [bv, n, fin]
    xpad = np.zeros((NCORES, NCORES * NPC, FIN), BF)
    xpad[:, :N, :] = xf.astype(BF)
    wT = np.ascontiguousarray(W.T.astype(BF))             # [64, 128]
    att2T = np.zeros((P, 2), np.float32)
    att2T[:, 0] = att[0, :, :F].reshape(-1)
    att2T[:, 1] = att[0, :, F:].reshape(-1)
    indsrc = np.zeros((P, 8), BF)
    inddst = np.zeros((P, 8), BF)
    for h in range(H):
        indsrc[h * F:(h + 1) * F, h] = 1.0
        inddst[h * F:(h + 1) * F, 4 + h] = 1.0
    in1 = []
    for c in range(NCORES):
        r0 = c * NPC
        sl = xpad[:, r0:r0 + NPC, :]                      # [8, NPC, 64]
        xT_c = np.ascontiguousarray(sl.transpose(2, 0, 1).reshape(FIN, -1))
        in1.append({"xT": xT_c, "wT": wT, "att2T": att2T,
                    "indsrc": indsrc, "inddst": inddst})
    r1 = run_bass_kernel_spmd(nc1, in1, core_ids=list(range(NCORES)), **RUN_KW)
    EXEC_TIMES["launch1"] = r1.exec_time_ns

    # ---- launch 2 ----
    rows = np.concatenate([r1.results[c]["rows"] for c in range(NCORES)])
    table = np.zeros((N + 1, TBL_COLS), BF)
    table[:N, :EA_COL + 4] = rows[:N, :EA_COL + 4]
    ee_full = np.concatenate([r1.results[c]["eeT"] for c in range(NCORES)],
                             axis=1)                      # [8, 10240]
    nc2 = _get(("l2", plan.cmax), build_launch2, plan.cmax)
    in2 = [{"table": table, "idx16": plan.idx16[c], "rel": plan.rel[c],
            "eeT": np.ascontiguousarray(ee_full[:, c * NPC:(c + 1) * NPC])}
           for c in range(NCORES)]
    r2 = run_bass_kernel_spmd(nc2, in2, core_ids=list(range(NCORES)), **RUN_KW)
    EXEC_TIMES["launch2"] = r2.exec_time_ns

    # ---- launch 3 ----
    nc3 = _get("l3", build_launch3)
    gatT = np.concatenate([r2.results[c]["gatT"] for c in range(NCORES)],
                          axis=2)                         # [8, 128, 10240]
    zparts = np.stack([r2.results[c]["zpart"][0] for c in range(NCORES)])
    wiT = np.ascontiguousarray(in_proj_w.T)               # [128, 384]
    woT = np.ascontiguousarray(out_proj_w.T.astype(BF))   # [128, 128]
    bi = np.ascontiguousarray(in_proj_b.reshape(1, 3 * D))
    bo = np.ascontiguousarray(out_proj_b.reshape(1, D))
    bb = np.ascontiguousarray(bias.reshape(1, D))
    in3 = []
    for c in range(NCORES):
        b, q = divmod(c, 4)
        xT4 = np.ascontiguousarray(
            gatT[b * V:(b + 1) * V, :, q * NQ:(q + 1) * NQ])  # [4, 128, 2500]
        in3.append({"xT4": xT4, "wiT": wiT, "bi": bi, "woT": woT,
                    "bo": bo, "bb": bb, "zparts": zparts})
    r3 = run_bass_kernel_spmd(nc3, in3, core_ids=list(range(NCORES)), **RUN_KW)
    EXEC_TIMES["launch3"] = r3.exec_time_ns

    out = np.empty((B, V, N, D), np.float32)
    for c in range(NCORES):
        b, q = divmod(c, 4)
        out[b, :, q * NQ:(q + 1) * NQ, :] = r3.results[c]["o"]
    return out


# revision 20
# speedup vs baseline: 3.5250x; 1.0590x over previous
"""Trainium2 Bass kernel for nn_GATv2Layer4View (GAT message passing + inter-view MHA).

Self-contained: kernel(**inputs) -> np.ndarray [2, 4, 10000, 128] float32.

Math (faithful to reference):
  scores[e,h] = mean_bv(s_src[bv, src[e], h] + s_dst[bv, dst[e], h])   (node-separable)
  w = softmax(scores, axis=0) over ALL edges per head
    = ea[src[e],h] * eb[dst[e],h] / Z[h],  ea = exp(ms_src), eb = exp(ms_dst),
      Z = sum_e ea[src[e]] * eb[dst[e]]
  gat[bv,d,:] = (eb[d]/Z) (*) sum_{e: dst=d} (ea[src[e]] (*) h[bv, src[e]])
  -> pure unweighted gather + scatter-add of table rows; eb applied at the end;
     1/Z[h] folded into the MHA in_proj weight rows (launch 3).

Launch 1 (node-sharded, 1280 nodes/core): h for all 8 (b,v), per-node score
  means -> ea/eb, and the packed gather-table rows
  [ea*h_bv0 .. ea*h_bv7 (8*128 bf16) | ea (4) | pad] = 1152 bf16 = 2304 B.
Launch 2 (dst-node-range sharded): dma_gather of its ~24k edges' mega-rows,
  one-hot scatter matmul into PSUM (8 batches + ea column share one one-hot),
  *eb finalize (unnormalized), partial-Z output.
Launch 3 ((b, node-quarter) sharded): inter-view MHA over V=4, bf16 compute,
  with sum(Z partials) -> 1/Z scaling folded into in_proj_w rows.
"""

import math
import numpy as np
import ml_dtypes

import concourse.bass as bass
import concourse.bacc as bacc
import concourse.mybir as mybir
import concourse.tile as tile
import concourse.bass_isa as bass_isa
from concourse.bass_utils import run_bass_kernel_spmd
from concourse.masks import make_identity

P = 128
NCORES = 8
B, V, N, FIN = 2, 4, 10000, 64
H, F = 4, 32
D = H * F                      # 128
E_RAW = 160000
NEG_SLOPE = 0.2

NPC = 1280                     # nodes per core (node-sharded launches 1/2)
TPC = NPC // P                 # 10 tiles per core
TBL_COLS = 1152                # bf16 cols: 8*128 h' + 4 ea + 124 pad = 2304 B
EA_COL = 8 * D                 # 1024
GATHER_GROUP = 8               # chunks per dma_gather (1024 rows)

NQ = N // 4                    # 2500 nodes per core in launch 3
CH = 125
NCH = NQ // CH                 # 20

FP32 = mybir.dt.float32
BF16 = mybir.dt.bfloat16
I16 = mybir.dt.int16
I32 = mybir.dt.int32

BF = ml_dtypes.bfloat16

RUN_KW = {}
EXEC_TIMES = {}


# --------------------------------------------------------------------------
# host-side edge preprocessing (per-core dst ranges, uniform chunk structure)
# --------------------------------------------------------------------------
class EdgePlan:
    pass


def prep_edges(edge_index: np.ndarray) -> EdgePlan:
    ei = np.asarray(edge_index)
    src = np.concatenate([ei[0].astype(np.int64), np.arange(N)])
    dst = np.concatenate([ei[1].astype(np.int64), np.arange(N)])
    order = np.argsort(dst, kind="stable")
    ss, ds = src[order], dst[order]

    n_tiles_total = NCORES * TPC  # 80 tile slots (the last ones may be empty)
    bounds = np.searchsorted(ds, np.minimum(np.arange(n_tiles_total + 1) * P, N))
    counts = np.diff(bounds)
    cmax = int(math.ceil(counts.max() / P))

    idx_all = np.full((NCORES, TPC * cmax * P), N, np.int64)   # pad -> zero row
    rel_all = np.full((NCORES, TPC * cmax * P), 200.0, np.float32)
    for c in range(NCORES):
        for t in range(TPC):
            g = c * TPC + t
            k = bounds[g + 1] - bounds[g]
            o = t * cmax * P
            idx_all[c, o:o + k] = ss[bounds[g]:bounds[g + 1]]
            rel_all[c, o:o + k] = ds[bounds[g]:bounds[g + 1]] - g * P
    plan = EdgePlan()
    plan.cmax = cmax
    plan.idx16 = [np.ascontiguousarray(idx_all[c].astype(np.int16)
                                       .reshape(-1, 16).T) for c in range(NCORES)]
    plan.rel = [np.ascontiguousarray(rel_all[c].reshape(-1, P).T.astype(np.float32))
                for c in range(NCORES)]
    return plan


# --------------------------------------------------------------------------
# launch 1: node-sharded. h for all 8 bv + score means + ea/eb + table rows
# --------------------------------------------------------------------------
def build_launch1():
    nc = bacc.Bacc("TRN2", target_bir_lowering=False, debug=False,
                   num_devices=NCORES)
    xT = nc.dram_tensor("xT", [FIN, NCORES * NPC], BF16, kind="ExternalInput")
    wT = nc.dram_tensor("wT", [FIN, D], BF16, kind="ExternalInput")
    att2T = nc.dram_tensor("att2T", [P, 2], FP32, kind="ExternalInput")
    indsrc = nc.dram_tensor("indsrc", [P, 8], BF16, kind="ExternalInput")
    inddst = nc.dram_tensor("inddst", [P, 8], BF16, kind="ExternalInput")
    rows_out = nc.dram_tensor("rows", [NPC, TBL_COLS], BF16, kind="ExternalOutput")
    ee_out = nc.dram_tensor("eeT", [8, NPC], FP32, kind="ExternalOutput")

    with tile.TileContext(nc) as tc:
        with tc.tile_pool(name="one", bufs=1) as one, \
             tc.tile_pool(name="sb", bufs=3) as sb, \
             tc.tile_pool(name="hb", bufs=16) as hb, \
             tc.tile_pool(name="pk", bufs=2) as pk, \
             tc.tile_pool(name="psA", bufs=2, space="PSUM") as psA, \
             tc.tile_pool(name="psB", bufs=2, space="PSUM") as psB, \
             tc.tile_pool(name="psS", bufs=2, space="PSUM") as psS:
            identity = one.tile([P, P], BF16)
            make_identity(nc, identity[:])
            idf32 = one.tile([8, 8], FP32)
            make_identity(nc, idf32[:])
            xT_sb = one.tile([FIN, NCORES * NPC], BF16)
            nc.sync.dma_start(xT_sb[:], xT.ap()[:])
            wT_sb = one.tile([FIN, D], BF16)
            nc.sync.dma_start(wT_sb[:], wT.ap()[:])
            att_sb = one.tile([P, 2], FP32)
            nc.sync.dma_start(att_sb[:], att2T.ap()[:])
            ind_sb = one.tile([P, 16], BF16)
            nc.sync.dma_start(ind_sb[:, 0:8], indsrc.ap()[:])
            nc.sync.dma_start(ind_sb[:, 8:16], inddst.ap()[:])
            ee_sb = one.tile([8, NPC], FP32)

            for t in range(TPC):
                n0 = t * P
                s_ps = psS.tile([8, P], FP32, tag="s")
                hn_list = []
                for bv in range(8):
                    hT_ps = psA.tile([P, P], FP32, tag="hT")
                    nc.tensor.matmul(hT_ps[:], wT_sb[:],
                                     xT_sb[:, bv * NPC + n0:bv * NPC + n0 + P],
                                     start=True, stop=True)
                    t1 = sb.tile([P, P], FP32, tag="t1")
                    nc.scalar.mul(t1[:], hT_ps[:], NEG_SLOPE)
                    hl = sb.tile([P, P], FP32, tag="hl")
                    nc.vector.tensor_tensor(out=hl[:], in0=hT_ps[:], in1=t1[:],
                                            op=mybir.AluOpType.max)
                    psrc = sb.tile([P, P], BF16, tag="psrc")
                    nc.vector.tensor_scalar_mul(psrc[:], hl[:], att_sb[:, 0:1])
                    pdst = sb.tile([P, P], BF16, tag="pdst")
                    nc.vector.tensor_scalar_mul(pdst[:], hl[:], att_sb[:, 1:2])
                    nc.tensor.matmul(s_ps[:], ind_sb[:, 0:8], psrc[:],
                                     start=(bv == 0), stop=False)
                    nc.tensor.matmul(s_ps[:], ind_sb[:, 8:16], pdst[:],
                                     start=False, stop=(bv == 7))
                    hn = hb.tile([P, P], BF16, tag="hn")
                    nc.scalar.copy(hn[:], hT_ps[:])
                    hn_list.append(hn)
                nc.scalar.activation(ee_sb[:, n0:n0 + P], s_ps[:],
                                     mybir.ActivationFunctionType.Exp,
                                     scale=1.0 / 8.0)
                ee_ps = psB.tile([P, 8], FP32, tag="eeT")
                nc.tensor.transpose(ee_ps[:, 0:8], ee_sb[:, n0:n0 + P],
                                    idf32[:])
                ea_nm = sb.tile([P, 4], FP32, tag="ea")
                nc.vector.tensor_copy(ea_nm[:], ee_ps[:, 0:4])
                packed = pk.tile([P, EA_COL + 4], BF16, tag="packed")
                for bv in range(8):
                    hT_node_ps = psB.tile([P, P], BF16, tag="hnode")
                    nc.tensor.transpose(hT_node_ps[:], hn_list[bv][:], identity[:])
                    nc.vector.tensor_tensor(
                        out=packed[:, bv * D:(bv + 1) * D].rearrange(
                            "p (h f) -> p h f", h=H),
                        in0=hT_node_ps[:].rearrange("p (h f) -> p h f", h=H),
                        in1=ea_nm[:, :, None].to_broadcast([P, H, F]),
                        op=mybir.AluOpType.mult)
                nc.vector.tensor_copy(packed[:, EA_COL:EA_COL + 4], ea_nm[:])
                nc.sync.dma_start(rows_out.ap()[n0:n0 + P, 0:EA_COL + 4],
                                  packed[:])
            nc.sync.dma_start(ee_out.ap()[:], ee_sb[:])
    nc.compile()
    return nc


# --------------------------------------------------------------------------
# launch 2: dst-range edge aggregation (all 8 bv at once)
# --------------------------------------------------------------------------
def build_launch2(cmax: int):
    n_chunks = TPC * cmax
    idx_cols = n_chunks * P // 16

    nc = bacc.Bacc("TRN2", target_bir_lowering=False, debug=False,
                   num_devices=NCORES)
    tbl_in = nc.dram_tensor("table", [N + 1, TBL_COLS], BF16, kind="ExternalInput")
    idx_in = nc.dram_tensor("idx16", [16, idx_cols], I16, kind="ExternalInput")
    rel_in = nc.dram_tensor("rel", [P, n_chunks], FP32, kind="ExternalInput")
    ee_in = nc.dram_tensor("eeT", [8, NPC], FP32, kind="ExternalInput")
    gat_out = nc.dram_tensor("gatT", [8, P, NPC], FP32, kind="ExternalOutput")
    z_out = nc.dram_tensor("zpart", [1, H], FP32, kind="ExternalOutput")

    groups = []
    c = 0
    while c < n_chunks:
        m = min(GATHER_GROUP, n_chunks - c)
        groups.append((c, m))
        c += m

    with tile.TileContext(nc) as tc:
        with tc.tile_pool(name="one", bufs=1) as one, \
             tc.tile_pool(name="sb", bufs=3) as sb, \
             tc.tile_pool(name="gp", bufs=3) as gp, \
             tc.tile_pool(name="ps", bufs=1, space="PSUM") as ps, \
             tc.tile_pool(name="acc", bufs=2, space="PSUM") as accp:
            identity = one.tile([P, P], FP32)
            make_identity(nc, identity[:])
            iota_i = one.tile([P, P], I32)
            nc.gpsimd.iota(iota_i[:], [[1, P]], channel_multiplier=0)
            iota_b = one.tile([P, P], BF16)
            nc.vector.tensor_copy(iota_b[:], iota_i[:])

            idx_sb = one.tile([P, idx_cols], I16)
            for r in range(8):
                nc.sync.dma_start(idx_sb[16 * r:16 * (r + 1), :], idx_in.ap()[:])
            rel_sb = one.tile([P, n_chunks], FP32)
            nc.sync.dma_start(rel_sb[:], rel_in.ap()[:])
            ee_sb = one.tile([8, NPC], FP32)
            nc.sync.dma_start(ee_sb[:], ee_in.ap()[:])

            gatT_sb = one.tile([P, 8 * NPC], FP32)   # [d, (bv, node)]
            zacc = one.tile([P, H], FP32)
            nc.vector.memset(zacc[:], 0.0)

            # precompute all one-hots + per-tile eb before the gather phase
            # (during gathers, SWDGE descriptor traffic slows DVE 5-9x)
            S_all = one.tile([P, n_chunks * P], BF16)
            for ci in range(n_chunks):
                nc.vector.tensor_scalar(
                    out=S_all[:, ci * P:(ci + 1) * P], in0=iota_b[:],
                    scalar1=rel_sb[:, ci:ci + 1], scalar2=None,
                    op0=mybir.AluOpType.is_equal)
            eb_all = one.tile([P, TPC * 4], FP32)
            for t in range(TPC):
                eb_ps = ps.tile([P, 8], FP32, tag="ebT")
                nc.tensor.transpose(eb_ps[:, 0:8], ee_sb[:, t * P:(t + 1) * P],
                                    identity[:8, :8])
                nc.vector.tensor_copy(eb_all[:, t * 4:(t + 1) * 4],
                                      eb_ps[:, 4:8])

            acc_ps = None
            for (c0, m) in groups:
                g = gp.tile([P, GATHER_GROUP, TBL_COLS], BF16, tag="g")
                nc.gpsimd.dma_gather(
                    out_ap=g[:, :m, :],
                    in_ap=tbl_in.ap()[:],
                    idxs_ap=idx_sb[:, c0 * 8:(c0 + m) * 8],
                    num_idxs=m * P,
                    num_idxs_reg=m * P,
                    elem_size=TBL_COLS,
                    single_packet=False,
                )
                for j in range(m):
                    ci = c0 + j
                    t, k = divmod(ci, cmax)
                    if k == 0:
                        acc_ps = accp.tile([P, EA_COL + 4], FP32, tag="acc")
                    S = S_all[:, ci * P:(ci + 1) * P]
                    # start=True zeroes the entire PSUM bank -> exactly one
                    # start per bank (one N=512 matmul per bank + ea)
                    for half in range(2):
                        nc.tensor.matmul(
                            acc_ps[:, half * 512:(half + 1) * 512], S,
                            g[:, j, half * 512:(half + 1) * 512],
                            start=(k == 0), stop=(k == cmax - 1),
                            skip_group_check=True)
                    nc.tensor.matmul(
                        acc_ps[:, EA_COL:EA_COL + 4], S,
                        g[:, j, EA_COL:EA_COL + 4],
                        start=(k == 0), stop=(k == cmax - 1),
                        skip_group_check=True)
                    if k == cmax - 1:
                        eb_nm = eb_all[:, t * 4:(t + 1) * 4]
                        for bv in range(8):
                            om = sb.tile([P, D], FP32, tag="om")
                            nc.vector.tensor_tensor(
                                out=om[:].rearrange("p (h f) -> p h f", h=H),
                                in0=acc_ps[:, bv * D:(bv + 1) * D].rearrange(
                                    "p (h f) -> p h f", h=H),
                                in1=eb_nm[:, :, None].to_broadcast([P, H, F]),
                                op=mybir.AluOpType.mult)
                            o_ps = ps.tile([P, P], FP32, tag="oT")
                            nc.tensor.transpose(o_ps[:], om[:], identity[:])
                            nc.vector.tensor_copy(
                                gatT_sb[:, bv * NPC + t * P:bv * NPC + (t + 1) * P],
                                o_ps[:])
                        zp = sb.tile([P, H], FP32, tag="zp")
                        nc.vector.tensor_tensor(
                            out=zp[:], in0=acc_ps[:, EA_COL:EA_COL + 4],
                            in1=eb_nm[:], op=mybir.AluOpType.mult)
                        nc.vector.tensor_tensor(
                            out=zacc[:], in0=zacc[:], in1=zp[:],
                            op=mybir.AluOpType.add)

            zred = one.tile([P, H], FP32)
            nc.gpsimd.partition_all_reduce(zred[:], zacc[:], channels=P,
                                           reduce_op=bass_isa.ReduceOp.add)
            nc.sync.dma_start(z_out.ap()[:], zred[0:1, :])
            nc.sync.dma_start(
                gat_out.ap().rearrange("v d n -> d v n"),
                gatT_sb[:].rearrange("d (v n) -> d v n", v=8))
    nc.compile()
    return nc


# --------------------------------------------------------------------------
# launch 3: inter-view MHA (bf16), 1/Z folded into the x scaling
# --------------------------------------------------------------------------
def build_launch3():
    hd = D // H      # 32
    nc = bacc.Bacc("TRN2", target_bir_lowering=False, debug=False,
                   num_devices=NCORES)
    xT4 = nc.dram_tensor("xT4", [V, P, NQ], FP32, kind="ExternalInput")
    wiT = nc.dram_tensor("wiT", [P, 3 * D], FP32, kind="ExternalInput")
    bi = nc.dram_tensor("bi", [1, 3 * D], FP32, kind="ExternalInput")
    woT = nc.dram_tensor("woT", [P, D], BF16, kind="ExternalInput")
    bo = nc.dram_tensor("bo", [1, D], FP32, kind="ExternalInput")
    bb = nc.dram_tensor("bb", [1, D], FP32, kind="ExternalInput")
    zparts = nc.dram_tensor("zparts", [8, H], FP32, kind="ExternalInput")
    o_out = nc.dram_tensor("o", [V, NQ, D], FP32, kind="ExternalOutput")

    with tile.TileContext(nc) as tc:
        with tc.tile_pool(name="one", bufs=1) as one, \
             tc.tile_pool(name="sb", bufs=3) as sb, \
             tc.tile_pool(name="qkvp", bufs=6) as qkvp, \
             tc.tile_pool(name="ps", bufs=2, space="PSUM") as ps, \
             tc.tile_pool(name="ps2", bufs=2, space="PSUM") as ps2:
            identity = one.tile([P, P], BF16)
            make_identity(nc, identity[:])
            zp_sb = one.tile([8, H], FP32)
            nc.sync.dma_start(zp_sb[:], zparts.ap()[:])
            zsum = one.tile([8, H], FP32)
            nc.gpsimd.partition_all_reduce(zsum[:], zp_sb[:], channels=8,
                                           reduce_op=bass_isa.ReduceOp.add)
            rz = one.tile([1, H], FP32)
            nc.vector.reciprocal(rz[:], zsum[0:1, :])
            rzrow = one.tile([1, D], FP32)
            nc.vector.tensor_copy(rzrow[:].rearrange("p (h f) -> p h f", h=H),
                                  rz[:, :, None].to_broadcast([1, H, hd]))
            idf = one.tile([1, 1], FP32)
            nc.vector.memset(idf[:], 1.0)
            rz_ps = ps.tile([P, 1], FP32, tag="rzT")
            nc.tensor.transpose(rz_ps[:, 0:1], rzrow[:], idf[:])
            rzcol = one.tile([P, 1], FP32)
            nc.vector.tensor_copy(rzcol[:], rz_ps[:, 0:1])
            x_sb = one.tile([P, V * NQ], FP32)
            nc.sync.dma_start(x_sb[:].rearrange("d (v n) -> d v n", v=V),
                              xT4.ap().rearrange("v d n -> d v n"))
            xb_sb = one.tile([P, V * NQ], BF16)
            nc.vector.tensor_scalar_mul(xb_sb[:], x_sb[:], rzcol[:, 0:1])

            wi_f = one.tile([P, 3 * D], FP32)
            nc.sync.dma_start(wi_f[:], wiT.ap()[:])
            wi_sb = one.tile([P, 3 * D], BF16)
            nc.vector.tensor_copy(wi_sb[:], wi_f[:])
            wo_sb = one.tile([P, D], BF16)
            nc.sync.dma_start(wo_sb[:], woT.ap()[:])
            bi_row = one.tile([1, 3 * D], FP32)
            nc.sync.dma_start(bi_row[:], bi.ap()[:])
            bi_rowb = one.tile([1, 3 * D], BF16)
            nc.vector.tensor_copy(bi_rowb[:], bi_row[:])
            bi_sb = one.tile([P, 3 * D], BF16)
            nc.gpsimd.partition_broadcast(bi_sb[:], bi_rowb[:])
            bo_row = one.tile([1, D], FP32)
            nc.sync.dma_start(bo_row[:], bo.ap()[:])
            bb_row = one.tile([1, D], FP32)
            nc.sync.dma_start(bb_row[:], bb.ap()[:])
            cb_row = one.tile([1, D], FP32)
            nc.vector.tensor_add(cb_row[:], bo_row[:], bb_row[:])
            cb_sb = one.tile([P, D], FP32)
            nc.gpsimd.partition_broadcast(cb_sb[:], cb_row[:])

            o_sb = one.tile([P, V * NCH * D], FP32)   # slot (q, c)

            # process chunks in PAIRS stacked along the free dim to amortize
            # the per-op DVE fixed cost (~58cy + errata bubble)
            for c2 in range(NCH // 2):
                qkv = []
                for v in range(V):
                    q2 = qkvp.tile([P, 2 * 3 * D], BF16, tag="qkv")
                    for ch in range(2):
                        c = c2 * 2 + ch
                        n0 = c * CH
                        q_ps = ps.tile([P, 3 * D], FP32, tag="qkv_ps")
                        nc.tensor.matmul(q_ps[:CH, :],
                                         xb_sb[:, v * NQ + n0:v * NQ + n0 + CH],
                                         wi_sb[:], start=True, stop=True)
                        qf = sb.tile([P, 3 * D], BF16, tag="qf")
                        nc.scalar.copy(qf[:CH, :], q_ps[:CH, :])
                        nc.gpsimd.tensor_tensor(
                            out=q2[:CH, ch * 384:(ch + 1) * 384],
                            in0=qf[:CH, :], in1=bi_sb[:CH, :],
                            op=mybir.AluOpType.add)
                    qkv.append(q2)
                L = sb.tile([P, 2 * V * H * V], FP32, tag="L")
                Lv = L[:].rearrange("p (c q h k) -> p c q h k", c=2, q=V, h=H)
                for q in range(V):
                    for k in range(V):
                        prod = sb.tile([P, 2 * D], BF16, tag="prod")
                        nc.vector.tensor_tensor(
                            out=prod[:CH, :].rearrange("p (c d) -> p c d", c=2),
                            in0=qkv[q][:CH, :].rearrange(
                                "p (c d) -> p c d", c=2)[:, :, 0:D],
                            in1=qkv[k][:CH, :].rearrange(
                                "p (c d) -> p c d", c=2)[:, :, D:2 * D],
                            op=mybir.AluOpType.mult)
                        nc.vector.tensor_reduce(
                            out=Lv[:CH, :, q, :, k],
                            in_=prod[:CH, :].rearrange(
                                "p (c h f) -> p c h f", c=2, h=H),
                            axis=mybir.AxisListType.X, op=mybir.AluOpType.add)
                M = sb.tile([P, 2 * V * H], FP32, tag="M")
                nc.vector.tensor_reduce(
                    out=M[:CH, :],
                    in_=L[:CH, :].rearrange("p (a k) -> p a k", k=V),
                    axis=mybir.AxisListType.X, op=mybir.AluOpType.max)
                Dm = sb.tile([P, 2 * V * H * V], FP32, tag="Dm")
                nc.vector.tensor_tensor(
                    out=Dm[:CH, :].rearrange("p (a k) -> p a k", k=V),
                    in0=L[:CH, :].rearrange("p (a k) -> p a k", k=V),
                    in1=M[:CH, :, None].to_broadcast([CH, 2 * V * H, V]),
                    op=mybir.AluOpType.subtract)
                Ex = sb.tile([P, 2 * V * H * V], FP32, tag="Ex")
                nc.scalar.activation(Ex[:CH, :], Dm[:CH, :],
                                     mybir.ActivationFunctionType.Exp,
                                     scale=1.0 / math.sqrt(hd))
                Ssum = sb.tile([P, 2 * V * H], FP32, tag="Ssum")
                nc.vector.tensor_reduce(
                    out=Ssum[:CH, :],
                    in_=Ex[:CH, :].rearrange("p (a k) -> p a k", k=V),
                    axis=mybir.AxisListType.X, op=mybir.AluOpType.add)
                R = sb.tile([P, 2 * V * H], FP32, tag="R")
                nc.vector.reciprocal(R[:CH, :], Ssum[:CH, :])
                A = sb.tile([P, 2 * V * H * V], BF16, tag="A")
                nc.vector.tensor_tensor(
                    out=A[:CH, :].rearrange("p (a k) -> p a k", k=V),
                    in0=Ex[:CH, :].rearrange("p (a k) -> p a k", k=V),
                    in1=R[:CH, :, None].to_broadcast([CH, 2 * V * H, V]),
                    op=mybir.AluOpType.mult)
                Av = A[:].rearrange("p (c q h k) -> p c q h k", c=2, q=V, h=H)
                for q in range(V):
                    O = sb.tile([P, 2 * D], BF16, tag="O")
                    Ov = O[:].rearrange("p (c h f) -> p c h f", c=2, h=H)
                    for k in range(V):
                        a_b = Av[:CH, :, q, :, k][:, :, :, None].to_broadcast(
                            [CH, 2, H, hd])
                        vv = qkv[k][:CH, :].rearrange(
                            "p (c x) -> p c x", c=2)[:, :, 2 * D:3 * D].rearrange(
                            "p c (h f) -> p c h f", h=H)
                        if k == 0:
                            nc.vector.tensor_tensor(out=Ov[:CH], in0=vv, in1=a_b,
                                                    op=mybir.AluOpType.mult)
                        else:
                            tmp = sb.tile([P, 2 * D], BF16, tag="avtmp")
                            tv = tmp[:].rearrange("p (c h f) -> p c h f",
                                                  c=2, h=H)
                            eng = nc.gpsimd if k != 1 else nc.vector
                            eng.tensor_tensor(out=tv[:CH], in0=vv, in1=a_b,
                                              op=mybir.AluOpType.mult)
                            nc.vector.tensor_tensor(out=Ov[:CH], in0=Ov[:CH],
                                                    in1=tv[:CH],
                                                    op=mybir.AluOpType.add)
                    for ch in range(2):
                        c = c2 * 2 + ch
                        ot_ps = ps2.tile([P, P], BF16, tag="ot")
                        nc.tensor.transpose(ot_ps[:, :CH],
                                            O[:CH, ch * D:(ch + 1) * D],
                                            identity[:CH, :CH])
                        oT = sb.tile([P, P], BF16, tag="oTsb")
                        nc.scalar.copy(oT[:, :CH], ot_ps[:, :CH])
                        f_ps = ps2.tile([P, D], FP32, tag="f")
                        nc.tensor.matmul(f_ps[:CH, :], oT[:, :CH], wo_sb[:],
                                         start=True, stop=True)
                        nc.vector.tensor_add(
                            o_sb[:CH, (q * NCH + c) * D:(q * NCH + c + 1) * D],
                            f_ps[:CH, :], cb_sb[:CH, :])

            nc.sync.dma_start(
                o_out.ap().rearrange("v (c p) d -> p v c d", p=CH),
                o_sb[:CH, :].rearrange("p (v c d) -> p v c d", v=V, c=NCH))
    nc.compile()
    return nc


# --------------------------------------------------------------------------
# host orchestration
# --------------------------------------------------------------------------
_cache = {}


def _get(name, builder, *args):
    if name not in _cache:
        _cache[name] = builder(*args)
    return _cache[name]


def kernel(x, W, att, in_proj_w, in_proj_b, out_proj_w, out_proj_b, bias,
           edge_index):
    x = np.asarray(x, np.float32)
    W = np.asarray(W, np.float32)
    att = np.asarray(att, np.float32)
    in_proj_w = np.asarray(in_proj_w, np.float32)
    in_proj_b = np.asarray(in_proj_b, np.float32)
    out_proj_w = np.asarray(out_proj_w, np.float32)
    out_proj_b = np.asarray(out_proj_b, np.float32)
    bias = np.asarray(bias, np.float32)

    plan_key = np.asarray(edge_index).tobytes()
    if ("plan", plan_key) not in _cache:
        _cache[("plan", plan_key)] = prep_edges(edge_index)
    plan = _cache[("plan", plan_key)]

    # ---- launch 1 ----
    nc1 = _get("l1", build_launch1)
    xf = x.reshape(NCORES, N, FIN)                        # [bv, n, fin]
    xpad = np.zeros((NCORES, NCORES * NPC, FIN), BF)
    xpad[:, :N, :] = xf.astype(BF)
    wT = np.ascontiguousarray(W.T.astype(BF))             # [64, 128]
    att2T = np.zeros((P, 2), np.float32)
    att2T[:, 0] = att[0, :, :F].reshape(-1)
    att2T[:, 1] = att[0, :, F:].reshape(-1)
    indsrc = np.zeros((P, 8), BF)
    inddst = np.zeros((P, 8), BF)
    for h in range(H):
        indsrc[h * F:(h + 1) * F, h] = 1.0
        inddst[h * F:(h + 1) * F, 4 + h] = 1.0
    in1 = []
    for c in range(NCORES):
        r0 = c * NPC
        sl = xpad[:, r0:r0 + NPC, :]                      # [8, NPC, 64]
        xT_c = np.ascontiguousarray(sl.transpose(2, 0, 1).reshape(FIN, -1))
        in1.append({"xT": xT_c, "wT": wT, "att2T": att2T,
                    "indsrc": indsrc, "inddst": inddst})
    r1 = run_bass_kernel_spmd(nc1, in1, core_ids=list(range(NCORES)), **RUN_KW)
    EXEC_TIMES["launch1"] = r1.exec_time_ns

    # ---- launch 2 ----
    rows = np.concatenate([r1.results[c]["rows"] for c in range(NCORES)])
    table = np.zeros((N + 1, TBL_COLS), BF)
    table[:N, :EA_COL + 4] = rows[:N, :EA_COL + 4]
    ee_full = np.concatenate([r1.results[c]["eeT"] for c in range(NCORES)],
                             axis=1)                      # [8, 10240]
    nc2 = _get(("l2", plan.cmax), build_launch2, plan.cmax)
    in2 = [{"table": table, "idx16": plan.idx16[c], "rel": plan.rel[c],
            "eeT": np.ascontiguousarray(ee_full[:, c * NPC:(c + 1) * NPC])}
           for c in range(NCORES)]
    r2 = run_bass_kernel_spmd(nc2, in2, core_ids=list(range(NCORES)), **RUN_KW)
    EXEC_TIMES["launch2"] = r2.exec_time_ns

    # ---- launch 3 ----
    nc3 = _get("l3", build_launch3)
    gatT = np.concatenate([r2.results[c]["gatT"] for c in range(NCORES)],
                          axis=2)                         # [8, 128, 10240]
    zparts = np.stack([r2.results[c]["zpart"][0] for c in range(NCORES)])
    wiT = np.ascontiguousarray(in_proj_w.T)               # [128, 384]
    woT = np.ascontiguousarray(out_proj_w.T.astype(BF))   # [128, 128]
    bi = np.ascontiguousarray(in_proj_b.reshape(1, 3 * D))
    bo = np.ascontiguousarray(out_proj_b.reshape(1, D))
    bb = np.ascontiguousarray(bias.reshape(1, D))
    in3 = []
    for c in range(NCORES):
        b, q = divmod(c, 4)
        xT4 = np.ascontiguousarray(
            gatT[b * V:(b + 1) * V, :, q * NQ:(q + 1) * NQ])  # [4, 128, 2500]
        in3.append({"xT4": xT4, "wiT": wiT, "bi": bi, "woT": woT,
                    "bo": bo, "bb": bb, "zparts": zparts})
    r3 = run_bass_kernel_spmd(nc3, in3, core_ids=list(range(NCORES)), **RUN_KW)
    EXEC_TIMES["launch3"] = r3.exec_time_ns

    out = np.empty((B, V, N, D), np.float32)
    for c in range(NCORES):
        b, q = divmod(c, 4)
        out[b, :, q * NQ:(q + 1) * NQ, :] = r3.results[c]["o"]
    return out


# revision 21
# speedup vs baseline: 3.6169x; 1.0261x over previous
"""Trainium2 Bass kernel for nn_GATv2Layer4View (GAT message passing + inter-view MHA).

Self-contained: kernel(**inputs) -> np.ndarray [2, 4, 10000, 128] float32.

Math (faithful to reference):
  scores[e,h] = mean_bv(s_src[bv, src[e], h] + s_dst[bv, dst[e], h])   (node-separable)
  w = softmax(scores, axis=0) over ALL edges per head
    = ea[src[e],h] * eb[dst[e],h] / Z[h],  ea = exp(ms_src), eb = exp(ms_dst),
      Z = sum_e ea[src[e]] * eb[dst[e]]
  gat[bv,d,:] = (eb[d]/Z) (*) sum_{e: dst=d} (ea[src[e]] (*) h[bv, src[e]])
  -> pure unweighted gather + scatter-add of table rows; eb applied at the end;
     1/Z[h] folded into the MHA in_proj weight rows (launch 3).

Launch 1 (node-sharded, 1280 nodes/core): h for all 8 (b,v), per-node score
  means -> ea/eb, and the packed gather-table rows
  [ea*h_bv0 .. ea*h_bv7 (8*128 bf16) | ea (4) | pad] = 1152 bf16 = 2304 B.
Launch 2 (dst-node-range sharded): dma_gather of its ~24k edges' mega-rows,
  one-hot scatter matmul into PSUM (8 batches + ea column share one one-hot),
  *eb finalize (unnormalized), partial-Z output.
Launch 3 ((b, node-quarter) sharded): inter-view MHA over V=4, bf16 compute,
  with sum(Z partials) -> 1/Z scaling folded into in_proj_w rows.
"""

import math
import numpy as np
import ml_dtypes

import concourse.bass as bass
import concourse.bacc as bacc
import concourse.mybir as mybir
import concourse.tile as tile
import concourse.bass_isa as bass_isa
from concourse.bass_utils import run_bass_kernel_spmd
from concourse.masks import make_identity

P = 128
NCORES = 8
B, V, N, FIN = 2, 4, 10000, 64
H, F = 4, 32
D = H * F                      # 128
E_RAW = 160000
NEG_SLOPE = 0.2

NPC = 1280                     # nodes per core (node-sharded launches 1/2)
TPC = NPC // P                 # 10 tiles per core
TBL_COLS = 1152                # bf16 cols: 8*128 h' + 4 ea + 124 pad = 2304 B
EA_COL = 8 * D                 # 1024
GATHER_GROUP = 8               # chunks per dma_gather (1024 rows)

NQ = N // 4                    # 2500 nodes per core in launch 3
CH = 125
NCH = NQ // CH                 # 20

FP32 = mybir.dt.float32
BF16 = mybir.dt.bfloat16
I16 = mybir.dt.int16
I32 = mybir.dt.int32

BF = ml_dtypes.bfloat16

RUN_KW = {}
EXEC_TIMES = {}


# --------------------------------------------------------------------------
# host-side edge preprocessing (per-core dst ranges, uniform chunk structure)
# --------------------------------------------------------------------------
class EdgePlan:
    pass


def prep_edges(edge_index: np.ndarray) -> EdgePlan:
    ei = np.asarray(edge_index)
    src = np.concatenate([ei[0].astype(np.int64), np.arange(N)])
    dst = np.concatenate([ei[1].astype(np.int64), np.arange(N)])
    order = np.argsort(dst, kind="stable")
    ss, ds = src[order], dst[order]

    n_tiles_total = NCORES * TPC  # 80 tile slots (the last ones may be empty)
    bounds = np.searchsorted(ds, np.minimum(np.arange(n_tiles_total + 1) * P, N))
    counts = np.diff(bounds)
    cmax = int(math.ceil(counts.max() / P))

    idx_all = np.full((NCORES, TPC * cmax * P), N, np.int64)   # pad -> zero row
    rel_all = np.full((NCORES, TPC * cmax * P), 200.0, np.float32)
    for c in range(NCORES):
        for t in range(TPC):
            g = c * TPC + t
            k = bounds[g + 1] - bounds[g]
            o = t * cmax * P
            idx_all[c, o:o + k] = ss[bounds[g]:bounds[g + 1]]
            rel_all[c, o:o + k] = ds[bounds[g]:bounds[g + 1]] - g * P
    plan = EdgePlan()
    plan.cmax = cmax
    plan.idx16 = [np.ascontiguousarray(idx_all[c].astype(np.int16)
                                       .reshape(-1, 16).T) for c in range(NCORES)]
    plan.rel = [np.ascontiguousarray(rel_all[c].reshape(-1, P).T.astype(np.float32))
                for c in range(NCORES)]
    return plan


# --------------------------------------------------------------------------
# launch 1: node-sharded. h for all 8 bv + score means + ea/eb + table rows
# --------------------------------------------------------------------------
def build_launch1():
    nc = bacc.Bacc("TRN2", target_bir_lowering=False, debug=False,
                   num_devices=NCORES)
    xT = nc.dram_tensor("xT", [FIN, NCORES * NPC], BF16, kind="ExternalInput")
    wT = nc.dram_tensor("wT", [FIN, D], BF16, kind="ExternalInput")
    att2T = nc.dram_tensor("att2T", [P, 2], FP32, kind="ExternalInput")
    indsrc = nc.dram_tensor("indsrc", [P, 8], BF16, kind="ExternalInput")
    inddst = nc.dram_tensor("inddst", [P, 8], BF16, kind="ExternalInput")
    rows_out = nc.dram_tensor("rows", [NPC, TBL_COLS], BF16, kind="ExternalOutput")
    ee_out = nc.dram_tensor("eeT", [8, NPC], FP32, kind="ExternalOutput")

    with tile.TileContext(nc) as tc:
        with tc.tile_pool(name="one", bufs=1) as one, \
             tc.tile_pool(name="sb", bufs=3) as sb, \
             tc.tile_pool(name="hb", bufs=16) as hb, \
             tc.tile_pool(name="pk", bufs=2) as pk, \
             tc.tile_pool(name="psA", bufs=2, space="PSUM") as psA, \
             tc.tile_pool(name="psB", bufs=2, space="PSUM") as psB, \
             tc.tile_pool(name="psS", bufs=2, space="PSUM") as psS:
            identity = one.tile([P, P], BF16)
            make_identity(nc, identity[:])
            idf32 = one.tile([8, 8], FP32)
            make_identity(nc, idf32[:])
            xT_sb = one.tile([FIN, NCORES * NPC], BF16)
            nc.sync.dma_start(xT_sb[:], xT.ap()[:])
            wT_sb = one.tile([FIN, D], BF16)
            nc.sync.dma_start(wT_sb[:], wT.ap()[:])
            att_sb = one.tile([P, 2], FP32)
            nc.sync.dma_start(att_sb[:], att2T.ap()[:])
            ind_sb = one.tile([P, 16], BF16)
            nc.sync.dma_start(ind_sb[:, 0:8], indsrc.ap()[:])
            nc.sync.dma_start(ind_sb[:, 8:16], inddst.ap()[:])
            ee_sb = one.tile([8, NPC], FP32)

            for t in range(TPC):
                n0 = t * P
                s_ps = psS.tile([8, P], FP32, tag="s")
                hn_list = []
                for bv in range(8):
                    hT_ps = psA.tile([P, P], FP32, tag="hT")
                    nc.tensor.matmul(hT_ps[:], wT_sb[:],
                                     xT_sb[:, bv * NPC + n0:bv * NPC + n0 + P],
                                     start=True, stop=True)
                    t1 = sb.tile([P, P], FP32, tag="t1")
                    nc.scalar.mul(t1[:], hT_ps[:], NEG_SLOPE)
                    hl = sb.tile([P, P], FP32, tag="hl")
                    nc.vector.tensor_tensor(out=hl[:], in0=hT_ps[:], in1=t1[:],
                                            op=mybir.AluOpType.max)
                    psrc = sb.tile([P, P], BF16, tag="psrc")
                    nc.vector.tensor_scalar_mul(psrc[:], hl[:], att_sb[:, 0:1])
                    pdst = sb.tile([P, P], BF16, tag="pdst")
                    nc.vector.tensor_scalar_mul(pdst[:], hl[:], att_sb[:, 1:2])
                    nc.tensor.matmul(s_ps[:], ind_sb[:, 0:8], psrc[:],
                                     start=(bv == 0), stop=False)
                    nc.tensor.matmul(s_ps[:], ind_sb[:, 8:16], pdst[:],
                                     start=False, stop=(bv == 7))
                    hn = hb.tile([P, P], BF16, tag="hn")
                    nc.scalar.copy(hn[:], hT_ps[:])
                    hn_list.append(hn)
                nc.scalar.activation(ee_sb[:, n0:n0 + P], s_ps[:],
                                     mybir.ActivationFunctionType.Exp,
                                     scale=1.0 / 8.0)
                ee_ps = psB.tile([P, 8], FP32, tag="eeT")
                nc.tensor.transpose(ee_ps[:, 0:8], ee_sb[:, n0:n0 + P],
                                    idf32[:])
                ea_nm = sb.tile([P, 4], FP32, tag="ea")
                nc.vector.tensor_copy(ea_nm[:], ee_ps[:, 0:4])
                packed = pk.tile([P, EA_COL + 4], BF16, tag="packed")
                for bv in range(8):
                    hT_node_ps = psB.tile([P, P], BF16, tag="hnode")
                    nc.tensor.transpose(hT_node_ps[:], hn_list[bv][:], identity[:])
                    nc.vector.tensor_tensor(
                        out=packed[:, bv * D:(bv + 1) * D].rearrange(
                            "p (h f) -> p h f", h=H),
                        in0=hT_node_ps[:].rearrange("p (h f) -> p h f", h=H),
                        in1=ea_nm[:, :, None].to_broadcast([P, H, F]),
                        op=mybir.AluOpType.mult)
                nc.vector.tensor_copy(packed[:, EA_COL:EA_COL + 4], ea_nm[:])
                nc.sync.dma_start(rows_out.ap()[n0:n0 + P, 0:EA_COL + 4],
                                  packed[:])
            nc.sync.dma_start(ee_out.ap()[:], ee_sb[:])
    nc.compile()
    return nc


# --------------------------------------------------------------------------
# launch 2: dst-range edge aggregation (all 8 bv at once)
# --------------------------------------------------------------------------
def build_launch2(cmax: int):
    n_chunks = TPC * cmax
    idx_cols = n_chunks * P // 16

    nc = bacc.Bacc("TRN2", target_bir_lowering=False, debug=False,
                   num_devices=NCORES)
    tbl_in = nc.dram_tensor("table", [N + 1, TBL_COLS], BF16, kind="ExternalInput")
    idx_in = nc.dram_tensor("idx16", [16, idx_cols], I16, kind="ExternalInput")
    rel_in = nc.dram_tensor("rel", [P, n_chunks], FP32, kind="ExternalInput")
    ee_in = nc.dram_tensor("eeT", [8, NPC], FP32, kind="ExternalInput")
    gat_out = nc.dram_tensor("gatT", [8, P, NPC], FP32, kind="ExternalOutput")
    z_out = nc.dram_tensor("zpart", [1, H], FP32, kind="ExternalOutput")

    groups = []
    c = 0
    while c < n_chunks:
        m = min(GATHER_GROUP, n_chunks - c)
        groups.append((c, m))
        c += m

    with tile.TileContext(nc) as tc:
        with tc.tile_pool(name="one", bufs=1) as one, \
             tc.tile_pool(name="sb", bufs=3) as sb, \
             tc.tile_pool(name="gp", bufs=3) as gp, \
             tc.tile_pool(name="ps", bufs=1, space="PSUM") as ps, \
             tc.tile_pool(name="acc", bufs=2, space="PSUM") as accp:
            identity = one.tile([P, P], FP32)
            make_identity(nc, identity[:])
            iota_i = one.tile([P, P], I32)
            nc.gpsimd.iota(iota_i[:], [[1, P]], channel_multiplier=0)
            iota_b = one.tile([P, P], BF16)
            nc.vector.tensor_copy(iota_b[:], iota_i[:])

            idx_sb = one.tile([P, idx_cols], I16)
            for r in range(8):
                nc.sync.dma_start(idx_sb[16 * r:16 * (r + 1), :], idx_in.ap()[:])
            rel_sb = one.tile([P, n_chunks], FP32)
            nc.sync.dma_start(rel_sb[:], rel_in.ap()[:])
            ee_sb = one.tile([8, NPC], FP32)
            nc.sync.dma_start(ee_sb[:], ee_in.ap()[:])

            gatT_sb = one.tile([P, 8 * NPC], FP32)   # [d, (bv, node)]
            zacc = one.tile([P, H], FP32)
            nc.vector.memset(zacc[:], 0.0)

            # precompute all one-hots + per-tile eb before the gather phase
            # (during gathers, SWDGE descriptor traffic slows DVE 5-9x)
            S_all = one.tile([P, n_chunks * P], BF16)
            for ci in range(n_chunks):
                nc.vector.tensor_scalar(
                    out=S_all[:, ci * P:(ci + 1) * P], in0=iota_b[:],
                    scalar1=rel_sb[:, ci:ci + 1], scalar2=None,
                    op0=mybir.AluOpType.is_equal)
            eb_all = one.tile([P, TPC * 4], FP32)
            for t in range(TPC):
                eb_ps = ps.tile([P, 8], FP32, tag="ebT")
                nc.tensor.transpose(eb_ps[:, 0:8], ee_sb[:, t * P:(t + 1) * P],
                                    identity[:8, :8])
                nc.vector.tensor_copy(eb_all[:, t * 4:(t + 1) * 4],
                                      eb_ps[:, 4:8])

            acc_ps = None
            for (c0, m) in groups:
                g = gp.tile([P, GATHER_GROUP, TBL_COLS], BF16, tag="g")
                nc.gpsimd.dma_gather(
                    out_ap=g[:, :m, :],
                    in_ap=tbl_in.ap()[:],
                    idxs_ap=idx_sb[:, c0 * 8:(c0 + m) * 8],
                    num_idxs=m * P,
                    num_idxs_reg=m * P,
                    elem_size=TBL_COLS,
                    single_packet=False,
                )
                for j in range(m):
                    ci = c0 + j
                    t, k = divmod(ci, cmax)
                    if k == 0:
                        acc_ps = accp.tile([P, EA_COL + 4], FP32, tag="acc")
                    S = S_all[:, ci * P:(ci + 1) * P]
                    # start=True zeroes the entire PSUM bank -> exactly one
                    # start per bank (one N=512 matmul per bank + ea)
                    for half in range(2):
                        nc.tensor.matmul(
                            acc_ps[:, half * 512:(half + 1) * 512], S,
                            g[:, j, half * 512:(half + 1) * 512],
                            start=(k == 0), stop=(k == cmax - 1),
                            skip_group_check=True)
                    nc.tensor.matmul(
                        acc_ps[:, EA_COL:EA_COL + 4], S,
                        g[:, j, EA_COL:EA_COL + 4],
                        start=(k == 0), stop=(k == cmax - 1),
                        skip_group_check=True)
                    if k == cmax - 1:
                        eb_nm = eb_all[:, t * 4:(t + 1) * 4]
                        for bv in range(8):
                            om = sb.tile([P, D], FP32, tag="om")
                            nc.vector.tensor_tensor(
                                out=om[:].rearrange("p (h f) -> p h f", h=H),
                                in0=acc_ps[:, bv * D:(bv + 1) * D].rearrange(
                                    "p (h f) -> p h f", h=H),
                                in1=eb_nm[:, :, None].to_broadcast([P, H, F]),
                                op=mybir.AluOpType.mult)
                            o_ps = ps.tile([P, P], FP32, tag="oT")
                            nc.tensor.transpose(o_ps[:], om[:], identity[:])
                            nc.vector.tensor_copy(
                                gatT_sb[:, bv * NPC + t * P:bv * NPC + (t + 1) * P],
                                o_ps[:])
                        zp = sb.tile([P, H], FP32, tag="zp")
                        nc.vector.tensor_tensor(
                            out=zp[:], in0=acc_ps[:, EA_COL:EA_COL + 4],
                            in1=eb_nm[:], op=mybir.AluOpType.mult)
                        nc.vector.tensor_tensor(
                            out=zacc[:], in0=zacc[:], in1=zp[:],
                            op=mybir.AluOpType.add)

            zred = one.tile([P, H], FP32)
            nc.gpsimd.partition_all_reduce(zred[:], zacc[:], channels=P,
                                           reduce_op=bass_isa.ReduceOp.add)
            nc.sync.dma_start(z_out.ap()[:], zred[0:1, :])
            nc.sync.dma_start(
                gat_out.ap().rearrange("v d n -> d v n"),
                gatT_sb[:].rearrange("d (v n) -> d v n", v=8))
    nc.compile()
    return nc


# --------------------------------------------------------------------------
# launch 3: inter-view MHA (bf16), 1/Z folded into the x scaling
# --------------------------------------------------------------------------
def build_launch3():
    hd = D // H      # 32
    nc = bacc.Bacc("TRN2", target_bir_lowering=False, debug=False,
                   num_devices=NCORES)
    xT4 = nc.dram_tensor("xT4", [V, P, NQ], FP32, kind="ExternalInput")
    wiT = nc.dram_tensor("wiT", [P, 3 * D], FP32, kind="ExternalInput")
    bi = nc.dram_tensor("bi", [1, 3 * D], FP32, kind="ExternalInput")
    woT = nc.dram_tensor("woT", [P, D], BF16, kind="ExternalInput")
    bo = nc.dram_tensor("bo", [1, D], FP32, kind="ExternalInput")
    bb = nc.dram_tensor("bb", [1, D], FP32, kind="ExternalInput")
    zparts = nc.dram_tensor("zparts", [8, H], FP32, kind="ExternalInput")
    o_out = nc.dram_tensor("o", [V, NQ, D], FP32, kind="ExternalOutput")

    with tile.TileContext(nc) as tc:
        with tc.tile_pool(name="one", bufs=1) as one, \
             tc.tile_pool(name="sb", bufs=3) as sb, \
             tc.tile_pool(name="qkvp", bufs=6) as qkvp, \
             tc.tile_pool(name="ps", bufs=2, space="PSUM") as ps, \
             tc.tile_pool(name="ps2", bufs=2, space="PSUM") as ps2:
            identity = one.tile([P, P], BF16)
            make_identity(nc, identity[:])
            zp_sb = one.tile([8, H], FP32)
            nc.sync.dma_start(zp_sb[:], zparts.ap()[:])
            zsum = one.tile([8, H], FP32)
            nc.gpsimd.partition_all_reduce(zsum[:], zp_sb[:], channels=8,
                                           reduce_op=bass_isa.ReduceOp.add)
            rz = one.tile([1, H], FP32)
            nc.vector.reciprocal(rz[:], zsum[0:1, :])
            rzrow = one.tile([1, D], FP32)
            nc.vector.tensor_copy(rzrow[:].rearrange("p (h f) -> p h f", h=H),
                                  rz[:, :, None].to_broadcast([1, H, hd]))
            idf = one.tile([1, 1], FP32)
            nc.vector.memset(idf[:], 1.0)
            rz_ps = ps.tile([P, 1], FP32, tag="rzT")
            nc.tensor.transpose(rz_ps[:, 0:1], rzrow[:], idf[:])
            rzcol = one.tile([P, 1], FP32)
            nc.vector.tensor_copy(rzcol[:], rz_ps[:, 0:1])
            x_sb = one.tile([P, V * NQ], FP32)
            nc.sync.dma_start(x_sb[:].rearrange("d (v n) -> d v n", v=V),
                              xT4.ap().rearrange("v d n -> d v n"))
            xb_sb = one.tile([P, V * NQ], BF16)
            nc.vector.tensor_scalar_mul(xb_sb[:], x_sb[:], rzcol[:, 0:1])

            wi_f = one.tile([P, 3 * D], FP32)
            nc.sync.dma_start(wi_f[:], wiT.ap()[:])
            wi_sb = one.tile([P, 3 * D], BF16)
            nc.vector.tensor_copy(wi_sb[:], wi_f[:])
            wo_sb = one.tile([P, D], BF16)
            nc.sync.dma_start(wo_sb[:], woT.ap()[:])
            bi_row = one.tile([1, 3 * D], FP32)
            nc.sync.dma_start(bi_row[:], bi.ap()[:])
            bi_rowb = one.tile([1, 3 * D], BF16)
            nc.vector.tensor_copy(bi_rowb[:], bi_row[:])
            bi_sb = one.tile([P, 3 * D], BF16)
            nc.gpsimd.partition_broadcast(bi_sb[:], bi_rowb[:])
            bo_row = one.tile([1, D], FP32)
            nc.sync.dma_start(bo_row[:], bo.ap()[:])
            bb_row = one.tile([1, D], FP32)
            nc.sync.dma_start(bb_row[:], bb.ap()[:])
            cb_row = one.tile([1, D], FP32)
            nc.vector.tensor_add(cb_row[:], bo_row[:], bb_row[:])
            cb_sb = one.tile([P, D], FP32)
            nc.gpsimd.partition_broadcast(cb_sb[:], cb_row[:])

            o_sb = one.tile([P, V * NCH * D], FP32)   # slot (q, c)

            # process chunks in groups of CW stacked along the free dim to
            # amortize the per-op DVE fixed cost (~58cy + errata bubble)
            CW = 4
            for c2 in range(NCH // CW):
                qkv = []
                for v in range(V):
                    q2 = qkvp.tile([P, CW * 3 * D], BF16, tag="qkv")
                    for ch in range(CW):
                        c = c2 * CW + ch
                        n0 = c * CH
                        q_ps = ps.tile([P, 3 * D], FP32, tag="qkv_ps")
                        nc.tensor.matmul(q_ps[:CH, :],
                                         xb_sb[:, v * NQ + n0:v * NQ + n0 + CH],
                                         wi_sb[:], start=True, stop=True)
                        qf = sb.tile([P, 3 * D], BF16, tag="qf")
                        nc.scalar.copy(qf[:CH, :], q_ps[:CH, :])
                        nc.gpsimd.tensor_tensor(
                            out=q2[:CH, ch * 384:(ch + 1) * 384],
                            in0=qf[:CH, :], in1=bi_sb[:CH, :],
                            op=mybir.AluOpType.add)
                    qkv.append(q2)
                L = sb.tile([P, CW * V * H * V], FP32, tag="L")
                Lv = L[:].rearrange("p (c q h k) -> p c q h k", c=CW, q=V, h=H)
                for q in range(V):
                    for k in range(V):
                        prod = sb.tile([P, CW * D], BF16, tag="prod")
                        nc.vector.tensor_tensor(
                            out=prod[:CH, :].rearrange("p (c d) -> p c d", c=CW),
                            in0=qkv[q][:CH, :].rearrange(
                                "p (c d) -> p c d", c=CW)[:, :, 0:D],
                            in1=qkv[k][:CH, :].rearrange(
                                "p (c d) -> p c d", c=CW)[:, :, D:2 * D],
                            op=mybir.AluOpType.mult)
                        nc.vector.tensor_reduce(
                            out=Lv[:CH, :, q, :, k],
                            in_=prod[:CH, :].rearrange(
                                "p (c h f) -> p c h f", c=CW, h=H),
                            axis=mybir.AxisListType.X, op=mybir.AluOpType.add)
                M = sb.tile([P, CW * V * H], FP32, tag="M")
                nc.vector.tensor_reduce(
                    out=M[:CH, :],
                    in_=L[:CH, :].rearrange("p (a k) -> p a k", k=V),
                    axis=mybir.AxisListType.X, op=mybir.AluOpType.max)
                Dm = sb.tile([P, CW * V * H * V], FP32, tag="Dm")
                nc.vector.tensor_tensor(
                    out=Dm[:CH, :].rearrange("p (a k) -> p a k", k=V),
                    in0=L[:CH, :].rearrange("p (a k) -> p a k", k=V),
                    in1=M[:CH, :, None].to_broadcast([CH, CW * V * H, V]),
                    op=mybir.AluOpType.subtract)
                Ex = sb.tile([P, CW * V * H * V], FP32, tag="Ex")
                nc.scalar.activation(Ex[:CH, :], Dm[:CH, :],
                                     mybir.ActivationFunctionType.Exp,
                                     scale=1.0 / math.sqrt(hd))
                Ssum = sb.tile([P, CW * V * H], FP32, tag="Ssum")
                nc.vector.tensor_reduce(
                    out=Ssum[:CH, :],
                    in_=Ex[:CH, :].rearrange("p (a k) -> p a k", k=V),
                    axis=mybir.AxisListType.X, op=mybir.AluOpType.add)
                R = sb.tile([P, CW * V * H], FP32, tag="R")
                nc.vector.reciprocal(R[:CH, :], Ssum[:CH, :])
                A = sb.tile([P, CW * V * H * V], BF16, tag="A")
                nc.vector.tensor_tensor(
                    out=A[:CH, :].rearrange("p (a k) -> p a k", k=V),
                    in0=Ex[:CH, :].rearrange("p (a k) -> p a k", k=V),
                    in1=R[:CH, :, None].to_broadcast([CH, CW * V * H, V]),
                    op=mybir.AluOpType.mult)
                Av = A[:].rearrange("p (c q h k) -> p c q h k", c=CW, q=V, h=H)
                for q in range(V):
                    O = sb.tile([P, CW * D], BF16, tag="O")
                    Ov = O[:].rearrange("p (c h f) -> p c h f", c=CW, h=H)
                    for k in range(V):
                        a_b = Av[:CH, :, q, :, k][:, :, :, None].to_broadcast(
                            [CH, CW, H, hd])
                        vv = qkv[k][:CH, :].rearrange(
                            "p (c x) -> p c x", c=CW)[:, :, 2 * D:3 * D].rearrange(
                            "p c (h f) -> p c h f", h=H)
                        if k == 0:
                            nc.vector.tensor_tensor(out=Ov[:CH], in0=vv, in1=a_b,
                                                    op=mybir.AluOpType.mult)
                        else:
                            tmp = sb.tile([P, CW * D], BF16, tag="avtmp")
                            tv = tmp[:].rearrange("p (c h f) -> p c h f",
                                                  c=CW, h=H)
                            eng = nc.gpsimd if k != 1 else nc.vector
                            eng.tensor_tensor(out=tv[:CH], in0=vv, in1=a_b,
                                              op=mybir.AluOpType.mult)
                            nc.vector.tensor_tensor(out=Ov[:CH], in0=Ov[:CH],
                                                    in1=tv[:CH],
                                                    op=mybir.AluOpType.add)
                    for ch in range(CW):
                        c = c2 * CW + ch
                        ot_ps = ps2.tile([P, P], BF16, tag="ot")
                        nc.tensor.transpose(ot_ps[:, :CH],
                                            O[:CH, ch * D:(ch + 1) * D],
                                            identity[:CH, :CH])
                        oT = sb.tile([P, P], BF16, tag="oTsb")
                        nc.scalar.copy(oT[:, :CH], ot_ps[:, :CH])
                        f_ps = ps2.tile([P, D], FP32, tag="f")
                        nc.tensor.matmul(f_ps[:CH, :], oT[:, :CH], wo_sb[:],
                                         start=True, stop=True)
                        nc.vector.tensor_add(
                            o_sb[:CH, (q * NCH + c) * D:(q * NCH + c + 1) * D],
                            f_ps[:CH, :], cb_sb[:CH, :])

            nc.sync.dma_start(
                o_out.ap().rearrange("v (c p) d -> p v c d", p=CH),
                o_sb[:CH, :].rearrange("p (v c d) -> p v c d", v=V, c=NCH))
    nc.compile()
    return nc


# --------------------------------------------------------------------------
# host orchestration
# --------------------------------------------------------------------------
_cache = {}


def _get(name, builder, *args):
    if name not in _cache:
        _cache[name] = builder(*args)
    return _cache[name]


def kernel(x, W, att, in_proj_w, in_proj_b, out_proj_w, out_proj_b, bias,
           edge_index):
    x = np.asarray(x, np.float32)
    W = np.asarray(W, np.float32)
    att = np.asarray(att, np.float32)
    in_proj_w = np.asarray(in_proj_w, np.float32)
    in_proj_b = np.asarray(in_proj_b, np.float32)
    out_proj_w = np.asarray(out_proj_w, np.float32)
    out_proj_b = np.asarray(out_proj_b, np.float32)
    bias = np.asarray(bias, np.float32)

    plan_key = np.asarray(edge_index).tobytes()
    if ("plan", plan_key) not in _cache:
        _cache[("plan", plan_key)] = prep_edges(edge_index)
    plan = _cache[("plan", plan_key)]

    # ---- launch 1 ----
    nc1 = _get("l1", build_launch1)
    xf = x.reshape(NCORES, N, FIN)                        # [bv, n, fin]
    xpad = np.zeros((NCORES, NCORES * NPC, FIN), BF)
    xpad[:, :N, :] = xf.astype(BF)
    wT = np.ascontiguousarray(W.T.astype(BF))             # [64, 128]
    att2T = np.zeros((P, 2), np.float32)
    att2T[:, 0] = att[0, :, :F].reshape(-1)
    att2T[:, 1] = att[0, :, F:].reshape(-1)
    indsrc = np.zeros((P, 8), BF)
    inddst = np.zeros((P, 8), BF)
    for h in range(H):
        indsrc[h * F:(h + 1) * F, h] = 1.0
        inddst[h * F:(h + 1) * F, 4 + h] = 1.0
    in1 = []
    for c in range(NCORES):
        r0 = c * NPC
        sl = xpad[:, r0:r0 + NPC, :]                      # [8, NPC, 64]
        xT_c = np.ascontiguousarray(sl.transpose(2, 0, 1).reshape(FIN, -1))
        in1.append({"xT": xT_c, "wT": wT, "att2T": att2T,
                    "indsrc": indsrc, "inddst": inddst})
    r1 = run_bass_kernel_spmd(nc1, in1, core_ids=list(range(NCORES)), **RUN_KW)
    EXEC_TIMES["launch1"] = r1.exec_time_ns

    # ---- launch 2 ----
    rows = np.concatenate([r1.results[c]["rows"] for c in range(NCORES)])
    table = np.zeros((N + 1, TBL_COLS), BF)
    table[:N, :EA_COL + 4] = rows[:N, :EA_COL + 4]
    ee_full = np.concatenate([r1.results[c]["eeT"] for c in range(NCORES)],
                             axis=1)                      # [8, 10240]
    nc2 = _get(("l2", plan.cmax), build_launch2, plan.cmax)
    in2 = [{"table": table, "idx16": plan.idx16[c], "rel": plan.rel[c],
            "eeT": np.ascontiguousarray(ee_full[:, c * NPC:(c + 1) * NPC])}
           for c in range(NCORES)]
    r2 = run_bass_kernel_spmd(nc2, in2, core_ids=list(range(NCORES)), **RUN_KW)
    EXEC_TIMES["launch2"] = r2.exec_time_ns

    # ---- launch 3 ----
    nc3 = _get("l3", build_launch3)
    gatT = np.concatenate([r2.results[c]["gatT"] for c in range(NCORES)],
                          axis=2)                         # [8, 128, 10240]
    zparts = np.stack([r2.results[c]["zpart"][0] for c in range(NCORES)])
    wiT = np.ascontiguousarray(in_proj_w.T)               # [128, 384]
    woT = np.ascontiguousarray(out_proj_w.T.astype(BF))   # [128, 128]
    bi = np.ascontiguousarray(in_proj_b.reshape(1, 3 * D))
    bo = np.ascontiguousarray(out_proj_b.reshape(1, D))
    bb = np.ascontiguousarray(bias.reshape(1, D))
    in3 = []
    for c in range(NCORES):
        b, q = divmod(c, 4)
        xT4 = np.ascontiguousarray(
            gatT[b * V:(b + 1) * V, :, q * NQ:(q + 1) * NQ])  # [4, 128, 2500]
        in3.append({"xT4": xT4, "wiT": wiT, "bi": bi, "woT": woT,
                    "bo": bo, "bb": bb, "zparts": zparts})
    r3 = run_bass_kernel_spmd(nc3, in3, core_ids=list(range(NCORES)), **RUN_KW)
    EXEC_TIMES["launch3"] = r3.exec_time_ns

    out = np.empty((B, V, N, D), np.float32)
    for c in range(NCORES):
        b, q = divmod(c, 4)
        out[b, :, q * NQ:(q + 1) * NQ, :] = r3.results[c]["o"]
    return out
